# revision 1
# baseline (speedup 1.0000x reference)
"""Class-routed autoencoder (moe_routing) Trainium2 kernel.

Strategy:
- The reference computes ALL 10 experts densely then gathers by label; we
  ROUTE instead: sort tokens by class on the host, split every class's tokens
  evenly across the 8 cores (class counts padded up to a multiple of 8 with
  dummy zero tokens), so every core runs an IDENTICAL program (SPMD) on
  N_core = sum_e ceil(c_e/8) tokens laid out as 10 contiguous single-class
  segments. Expert layers slice the right weight block per segment at
  compile time; no gather/scatter on device.
- Everything runs feature-major ([features, tokens]): weights are the
  stationary matmul operand as-is (out = W.T @ x_fm), the batch is the
  moving/free dim, and per-feature bias + ReLU + PSUM->SBUF evacuation fuse
  into one scalar-engine activation op (bias is per-partition).
- Matmuls run in bf16 (weights/x converted on host; fp32 PSUM accumulate,
  biases added in fp32). Final layer output is fp32.
- enc1+enc2 and dec1+dec2 are fused per chunk so the big hidden activations
  (2048-dim) never leave SBUF; h2/e1/e2 (bottleneck dims) are SBUF-resident
  full width. Chunks are a balanced split of N_core with sizes <= 512 (one
  PSUM bank per matmul).
- Expert execution is interleaved into the encoder chunk loop: each expert
  runs as soon as the encoder chunks covering its column segment are done,
  so the expert-weight DMA stream (5-deep ring, issued with no false
  dependencies) hides entirely under encoder compute. Decoder weights
  prefetch during the expert tail via the gpsimd sequencer.
- Host: permute+transpose x, run 8 cores, inverse-permute the output.
"""

import ml_dtypes
import numpy as np

import concourse.bass as bass
import concourse.mybir as mybir
import concourse.tile as tile
from concourse import bacc
from concourse.bass_utils import run_bass_kernel_spmd

N_CORES = 8
N_CLS = 10
D_IN, D_H, D_BOT, D_EXP = 1024, 2048, 512, 1024

F32 = mybir.dt.float32
BF16 = mybir.dt.bfloat16
RELU = mybir.ActivationFunctionType.Relu
IDENT = mybir.ActivationFunctionType.Identity

CHUNK = 512  # max matmul moving-operand (free dim) size: one PSUM bank fp32


def _chunks(n, step=CHUNK):
    """Balanced split of n into ceil(n/step) near-equal pieces (all <= step)."""
    nch = -(-n // step)
    base, rem = divmod(n, nch)
    out = []
    s = 0
    for i in range(nch):
        sz = base + (1 if i < rem else 0)
        out.append((s, sz))
        s += sz
    return out


def _build(n_seg, n_core):
    """Build the SPMD program for per-class-per-core counts n_seg (sum=n_core)."""
    nc = bacc.Bacc()

    xt = nc.dram_tensor("xt", [D_IN, n_core], BF16, kind="ExternalInput")
    w1 = nc.dram_tensor("w1", [D_IN, D_H], BF16, kind="ExternalInput")
    b1 = nc.dram_tensor("b1", [128, D_H // 128], F32, kind="ExternalInput")
    w2 = nc.dram_tensor("w2", [D_H, D_BOT], BF16, kind="ExternalInput")
    b2 = nc.dram_tensor("b2", [128, D_BOT // 128], F32, kind="ExternalInput")
    ew1 = nc.dram_tensor("ew1", [N_CLS, D_BOT, D_EXP], BF16, kind="ExternalInput")
    eb1 = nc.dram_tensor("eb1", [128, N_CLS, D_EXP // 128], F32, kind="ExternalInput")
    ew2 = nc.dram_tensor("ew2", [N_CLS, D_EXP, D_BOT], BF16, kind="ExternalInput")
    eb2 = nc.dram_tensor("eb2", [128, N_CLS, D_BOT // 128], F32, kind="ExternalInput")
    dw1 = nc.dram_tensor("dw1", [D_BOT, D_H], BF16, kind="ExternalInput")
    db1 = nc.dram_tensor("db1", [128, D_H // 128], F32, kind="ExternalInput")
    dw2 = nc.dram_tensor("dw2", [D_H, D_IN], BF16, kind="ExternalInput")
    db2 = nc.dram_tensor("db2", [128, D_IN // 128], F32, kind="ExternalInput")
    out = nc.dram_tensor("out", [D_IN, n_core], F32, kind="ExternalOutput")

    segs = []  # (class e, col start, col len)
    s = 0
    for e in range(N_CLS):
        if n_seg[e] > 0:
            segs.append((e, s, n_seg[e]))
            s += n_seg[e]
    chunks = _chunks(n_core)
    XC_BUFS = (D_IN // 128) * min(len(chunks), 4)

    KT1, MT1 = D_IN // 128, D_H // 128     # enc1: 8, 16
    KT2, MT2 = D_H // 128, D_BOT // 128    # enc2: 16, 4
    KE1, ME1 = D_BOT // 128, D_EXP // 128  # exp1: 4, 8
    KE2, ME2 = D_EXP // 128, D_BOT // 128  # exp2: 8, 4
    KD1, MD1 = D_BOT // 128, D_H // 128    # dec1: 4, 16
    KD2, MD2 = D_H // 128, D_IN // 128     # dec2: 16, 8

    with tile.TileContext(nc) as tc:
        p_const = tc.alloc_tile_pool(name="const", bufs=1)
        p_ps = tc.alloc_tile_pool(name="ps", bufs=8, space="PSUM")

        def bias_tile(h, tag, shape):
            t = p_const.tile(shape, F32, tag=tag, name=tag)
            nc.sync.dma_start(out=t, in_=h[:])
            return t

        # bottleneck activations, SBUF-resident at full width
        p_e2 = tc.alloc_tile_pool(name="e2", bufs=1)
        p_h2 = tc.alloc_tile_pool(name="h2", bufs=1)
        e2_t = [p_e2.tile([128, n_core], BF16, tag=f"e2_{m}", name=f"e2_{m}")
                for m in range(D_BOT // 128)]
        h2_t = [p_h2.tile([128, n_core], BF16, tag=f"h2_{m}", name=f"h2_{m}")
                for m in range(D_BOT // 128)]

        # Expert pool is allocated BEFORE the encoder pool: its space never
        # overlaps encoder tiles, so expert-weight DMAs carry no false deps
        # and prefetch during the encoder phase.
        EW_BUFS = 5
        ECHUNK = 256
        p_exp = tc.alloc_tile_pool(name="exp", bufs=1)
        # e1 lives only within one expert's exp1->exp2 pair: a 2-deep ring of
        # [128, ME1, ECHUNK] tiles instead of full-width buffers.
        e1_ring = [p_exp.tile([128, D_EXP // 128, ECHUNK], BF16, tag=f"e1r_{i}",
                              name=f"e1r_{i}") for i in range(3)]
        # Preallocate the expert-weight ring HERE (before the encoder pool) so
        # the slots live below the encoder arena and the weight DMAs carry no
        # false WAR deps on encoder compute -> they prefetch from t~0.
        ew1_ring = [p_exp.tile([128, KE1, D_EXP], BF16, tag=f"ew1_{i}",
                               name=f"ew1_{i}") for i in range(EW_BUFS)]
        ew2_ring = [p_exp.tile([128, KE2, D_BOT], BF16, tag=f"ew2_{i}",
                               name=f"ew2_{i}") for i in range(EW_BUFS)]

        # ---------------- encoder (fused enc1+enc2 per chunk) -----------------
        p_enc = tc.alloc_tile_pool(name="enc", bufs=1)

        # Weight tiles are loaded with few, large DMAs (the sync sequencer
        # costs ~0.65us per DMA trigger, which dominated startup latency), but
        # split k-wise into separate tiles so matmuls can start as soon as the
        # first slice lands.
        def load_w(dram_h, pool, tag, kt, mt_cols, ksplit):
            tiles = []
            per = kt // ksplit
            for i in range(ksplit):
                t = pool.tile([128, per, mt_cols], BF16, tag=f"{tag}{i}",
                              name=f"{tag}{i}")
                nc.sync.dma_start(
                    out=t,
                    in_=dram_h[i * per * 128:(i + 1) * per * 128, :]
                    .rearrange("(a p) n -> p a n", p=128))
                tiles.append(t)
            return lambda k: tiles[k // per][:, k % per, :]

        def load_xc(c0, cl):
            t = p_enc.tile([128, KT1, CHUNK], BF16, tag="xc", name="xc", bufs=3)
            nc.sync.dma_start(
                out=t[:, :, :cl],
                in_=xt[:, c0:c0 + cl].rearrange("(a p) n -> p a n", p=128))
            return t

        # x chunk 0 and the first W1 slices go first on the sync sequencer.
        xc = load_xc(*chunks[0])
        w1_at = load_w(w1, p_enc, "w1_", KT1, D_H, 4)
        w2_at = None

        # bias loads issue after the critical-path x/W1 DMAs (first use ~25us in)
        b1_t = bias_tile(b1, "b1", [128, MT1])
        b2_t = bias_tile(b2, "b2", [128, MT2])
        eb1_t = bias_tile(eb1, "eb1", [128, N_CLS, ME1])
        eb2_t = bias_tile(eb2, "eb2", [128, N_CLS, ME2])
        db1_t = bias_tile(db1, "db1", [128, MD1])
        db2_t = bias_tile(db2, "db2", [128, MD2])

        # experts are emitted as soon as the encoder chunks covering their
        # column segment are done: their compute absorbs expert-weight DMA
        # latency, and the PE never waits on the weight stream at phase end.
        seg_queue = list(segs)
        exp_counter = [0]
        unit_ctr = [0]
        pend = [None]  # exp2 of each unit is delayed one unit behind its exp1

        def emit_exp1(u):
            e, a, al, slot, ew1_t, _ = u
            e1c = e1_ring[slot]
            for m in range(ME1):
                ps = p_ps.tile([128, al], F32, tag="ps", name="ps")
                for k in range(KE1):
                    nc.tensor.matmul(ps, ew1_t[:, k, m * 128:(m + 1) * 128],
                                     h2_t[k][:, a:a + al],
                                     start=(k == 0), stop=(k == KE1 - 1))
                # bias+relu on the (idle) vector engine: keeps PSUM
                # evacuation off the scalar engine's critical path
                nc.vector.tensor_scalar(
                    out=e1c[:, m, :al], in0=ps,
                    scalar1=eb1_t[:, e, m:m + 1], scalar2=0.0,
                    op0=mybir.AluOpType.add, op1=mybir.AluOpType.max)

        def emit_exp2(u):
            e, a, al, slot, _, ew2_t = u
            e1c = e1_ring[slot]
            for m in range(ME2):
                ps = p_ps.tile([128, al], F32, tag="ps", name="ps")
                for k in range(KE2):
                    nc.tensor.matmul(ps, ew2_t[:, k, m * 128:(m + 1) * 128],
                                     e1c[:, k, :al],
                                     start=(k == 0), stop=(k == KE2 - 1))
                nc.scalar.activation(out=e2_t[m][:, a:a + al], in_=ps,
                                     func=RELU, bias=eb2_t[:, e, m:m + 1],
                                     scale=1.0)

        def emit_expert(e, s0, sl):
            # exp1(unit i) then exp2(unit i-1): exp1's PSUM evacuations (DVE)
            # overlap the next unit's exp1 matmuls instead of stalling the PE
            ei = exp_counter[0]
            exp_counter[0] += 1
            ew1_t = ew1_ring[ei % EW_BUFS]
            nc.sync.dma_start(
                out=ew1_t, in_=ew1[e].rearrange("(a p) n -> p a n", p=128))
            ew2_t = ew2_ring[ei % EW_BUFS]
            nc.sync.dma_start(
                out=ew2_t, in_=ew2[e].rearrange("(a p) n -> p a n", p=128))
            for c0, cl in _chunks(sl, ECHUNK):
                u = (e, s0 + c0, cl, unit_ctr[0] % 3, ew1_t, ew2_t)
                unit_ctr[0] += 1
                emit_exp1(u)
                if pend[0] is not None:
                    emit_exp2(pend[0])
                pend[0] = u

        for ci, (c0, cl) in enumerate(chunks):
            if ci > 0:
                xc = load_xc(c0, cl)
            h1c = []
            for m in range(MT1):
                ps = p_ps.tile([128, cl], F32, tag="ps", name="ps")
                for k in range(KT1):
                    nc.tensor.matmul(ps, w1_at(k)[:, m * 128:(m + 1) * 128],
                                     xc[:, k, :cl],
                                     start=(k == 0), stop=(k == KT1 - 1))
                h = p_enc.tile([128, CHUNK], BF16, tag="h1c", name="h1c",
                               bufs=MT1)
                nc.scalar.activation(out=h[:, :cl], in_=ps, func=RELU,
                                     bias=b1_t[:, m:m + 1], scale=1.0)
                h1c.append(h)
            if ci == 0:
                w2_at = load_w(w2, p_enc, "w2_", KT2, D_BOT, 2)
            for m in range(MT2):
                ps = p_ps.tile([128, cl], F32, tag="ps", name="ps")
                for k in range(KT2):
                    nc.tensor.matmul(ps, w2_at(k)[:, m * 128:(m + 1) * 128],
                                     h1c[k][:, :cl],
                                     start=(k == 0), stop=(k == KT2 - 1))
                nc.scalar.activation(out=h2_t[m][:, c0:c0 + cl], in_=ps, func=RELU,
                                     bias=b2_t[:, m:m + 1], scale=1.0)
            # run every expert whose segment is fully covered by done chunks
            chunk_end = c0 + cl
            while seg_queue and seg_queue[0][1] + seg_queue[0][2] <= chunk_end:
                e, s0, sl = seg_queue.pop(0)
                emit_expert(e, s0, sl)

        for e, s0, sl in seg_queue:
            emit_expert(e, s0, sl)
        if pend[0] is not None:
            emit_exp2(pend[0])
            pend[0] = None

        p_enc.release()

        # Decoder weights: gpsimd-triggered (waits on freed encoder space must
        # not block the sync sequencer), streaming during the expert tail.
        p_dec = tc.alloc_tile_pool(name="dec", bufs=1)
        dw1_tile = p_dec.tile([128, KD1, D_H], BF16, tag="dw1", name="dw1")
        nc.gpsimd.dma_start(out=dw1_tile,
                            in_=dw1[:].rearrange("(a p) n -> p a n", p=128))
        dw1_at = lambda k: dw1_tile[:, k, :]
        dw2_tiles = []
        for i in range(2):
            t = p_dec.tile([128, KD2 // 2, D_IN], BF16, tag=f"dw2_{i}",
                           name=f"dw2_{i}")
            nc.gpsimd.dma_start(
                out=t,
                in_=dw2[i * 8 * 128:(i + 1) * 8 * 128, :]
                .rearrange("(a p) n -> p a n", p=128))
            dw2_tiles.append(t)
        dw2_at = lambda k: dw2_tiles[k // 8][:, k % 8, :]

        # ---------------- decoder (fused dec1+dec2 per chunk) -----------------
        for c0, cl in chunks:
            d1c = []
            for m in range(MD1):
                ps = p_ps.tile([128, cl], F32, tag="ps", name="ps")
                for k in range(KD1):
                    nc.tensor.matmul(ps, dw1_at(k)[:, m * 128:(m + 1) * 128],
                                     e2_t[k][:, c0:c0 + cl],
                                     start=(k == 0), stop=(k == KD1 - 1))
                d = p_dec.tile([128, CHUNK], BF16, tag="d1c", name="d1c",
                               bufs=MD1)
                nc.scalar.activation(out=d[:, :cl], in_=ps, func=RELU,
                                     bias=db1_t[:, m:m + 1], scale=1.0)
                d1c.append(d)
            for m in range(MD2):
                ps = p_ps.tile([128, cl], F32, tag="ps", name="ps")
                for k in range(KD2):
                    nc.tensor.matmul(ps, dw2_at(k)[:, m * 128:(m + 1) * 128],
                                     d1c[k][:, :cl],
                                     start=(k == 0), stop=(k == KD2 - 1))
                o_t = p_dec.tile([128, CHUNK], F32, tag="o", name="o", bufs=4)
                nc.scalar.activation(out=o_t[:, :cl], in_=ps, func=IDENT,
                                     bias=db2_t[:, m:m + 1], scale=1.0)
                nc.sync.dma_start(out=out[m * 128:(m + 1) * 128, c0:c0 + cl],
                                  in_=o_t[:, :cl])

        p_dec.release()
        p_exp.release()
        p_h2.release()
        p_e2.release()
        p_ps.release()
        p_const.release()

    nc.finalize()
    return nc


_CACHE = {}


def _get_nc(n_seg, n_core):
    key = tuple(n_seg)
    if key not in _CACHE:
        _CACHE[key] = _build(n_seg, n_core)
    return _CACHE[key]


def _bf16(a):
    return np.ascontiguousarray(np.asarray(a, np.float32).astype(ml_dtypes.bfloat16))


def _f32(a):
    return np.ascontiguousarray(np.asarray(a, np.float32))


def _bias_fm(b, mt):
    """[mt*128] -> [128, mt] feature-major (partition-contiguous) layout."""
    return np.ascontiguousarray(np.asarray(b, np.float32).reshape(mt, 128).T)


def _ebias_fm(b, mt):
    """[N_CLS, mt*128] -> [128, N_CLS, mt]."""
    a = np.asarray(b, np.float32).reshape(N_CLS, mt, 128)
    return np.ascontiguousarray(a.transpose(2, 0, 1))


def kernel(x, labels, W1, b1, W2, b2, EW1, Eb1, EW2, Eb2, DW1, Db1, DW2, Db2):
    x = np.asarray(x, dtype=np.float32)
    labels_np = np.asarray(labels).astype(np.int64)
    B = x.shape[0]

    counts = np.bincount(labels_np, minlength=N_CLS)
    n_seg = [int(-(-int(c) // N_CORES)) for c in counts]  # ceil(c/8)
    n_core = int(sum(n_seg))

    # assign tokens: class e sorted tokens padded to 8*n_seg[e], row j -> core j
    order = np.argsort(labels_np, kind="stable")
    idx_by_class = np.split(order, np.cumsum(counts)[:-1])
    core_tok = np.full((N_CORES, n_core), -1, dtype=np.int64)
    off = 0
    for e in range(N_CLS):
        ne = n_seg[e]
        if ne == 0:
            continue
        padded = np.full(N_CORES * ne, -1, dtype=np.int64)
        padded[:counts[e]] = idx_by_class[e]
        core_tok[:, off:off + ne] = padded.reshape(N_CORES, ne)
        off += ne

    weights = {
        "w1": _bf16(W1), "b1": _bias_fm(b1, D_H // 128),
        "w2": _bf16(W2), "b2": _bias_fm(b2, D_BOT // 128),
        "ew1": _bf16(EW1), "eb1": _ebias_fm(Eb1, D_EXP // 128),
        "ew2": _bf16(EW2), "eb2": _ebias_fm(Eb2, D_BOT // 128),
        "dw1": _bf16(DW1), "db1": _bias_fm(Db1, D_H // 128),
        "dw2": _bf16(DW2), "db2": _bias_fm(Db2, D_IN // 128),
    }

    x_bf = x.astype(ml_dtypes.bfloat16)
    in_maps = []
    for j in range(N_CORES):
        ids = core_tok[j]
        valid = ids >= 0
        xc = np.zeros((n_core, D_IN), dtype=ml_dtypes.bfloat16)
        xc[valid] = x_bf[ids[valid]]
        im = {"xt": np.ascontiguousarray(xc.T)}
        im.update(weights)
        in_maps.append(im)

    nc = _get_nc(n_seg, n_core)
    res = run_bass_kernel_spmd(nc, in_maps, core_ids=list(range(N_CORES)))

    out = np.empty((B, D_IN), dtype=np.float32)
    for j in range(N_CORES):
        oc = res.results[j]["out"]  # [D_IN, n_core]
        ids = core_tok[j]
        valid = ids >= 0
        out[ids[valid]] = oc.T[valid]
    return out



# revision 2
# speedup vs baseline: 1.0084x; 1.0084x over previous
"""Class-routed autoencoder (moe_routing) Trainium2 kernel.

Strategy:
- The reference computes ALL 10 experts densely then gathers by label; we
  ROUTE instead: sort tokens by class on the host, split every class's tokens
  evenly across the 8 cores (class counts padded up to a multiple of 8 with
  dummy zero tokens), so every core runs an IDENTICAL program (SPMD) on
  N_core = sum_e ceil(c_e/8) tokens laid out as 10 contiguous single-class
  segments. Expert layers slice the right weight block per segment at
  compile time; no gather/scatter on device.
- Everything runs feature-major ([features, tokens]): weights are the
  stationary matmul operand as-is (out = W.T @ x_fm), the batch is the
  moving/free dim, and per-feature bias + ReLU + PSUM->SBUF evacuation fuse
  into one scalar-engine activation op (bias is per-partition).
- Matmuls run in bf16 (fp32 PSUM accumulate, biases added in fp32).
- The kernel is PE-row-bound (~192us of row streaming at 2.4GHz); the
  optimization work is in eliminating non-streaming time:
  * ~16 warmup matmuls on zeroed tiles run during the initial DMA wait so
    the PE p-state ramp (0.65->2.4GHz over ~3us) completes before real data
    arrives.
  * The first encoder chunk (512 cols) runs enc1 k-OUTER over 8 PSUM banks
    per m-half, so the first matmul needs only the first x k-pair (0.26MB)
    + the first half W1 k-slice (0.25MB) instead of x-chunk + 1MB W1 slice.
    W1 is split into 9 DMAs interleaved with the x k-pair DMAs in
    need-order; W2/xc1/xc2 stream behind them.
  * All 6 bias vectors are packed into ONE [128,164] f32 dram tensor (one
    DMA trigger instead of six; the sync sequencer pays ~1.3us per trigger).
  * Each expert's two weight matrices are packed into ONE [128,8192] bf16
    tensor (one trigger per expert), prefetched through a 4-deep ring whose
    slots are allocated below the encoder arena (no false WAR deps).
  * Decoder dw1 is preallocated below the encoder arena and its sync DMA
    triggers right after the last expert weights, so dec1 never waits
    (the baseline had a ~2us PE gap here); dw2 streams via the gpsimd
    sequencer into released encoder space during the expert tail.
  * Decoder chunks are [~450, ~450, 128]: the small last chunk shrinks the
    post-last-matmul activation+output-DMA tail.
- Expert execution is interleaved into the encoder chunk loop: each expert
  runs as soon as the encoder chunks covering its column segment are done,
  so the expert-weight DMA stream hides entirely under encoder compute.
- Host: permute+transpose x, run 8 cores, inverse-permute the output.
"""

import ml_dtypes
import numpy as np

import concourse.bass as bass
import concourse.mybir as mybir
import concourse.tile as tile
from concourse import bacc
from concourse.bass_utils import run_bass_kernel_spmd

N_CORES = 8
N_CLS = 10
D_IN, D_H, D_BOT, D_EXP = 1024, 2048, 512, 1024

F32 = mybir.dt.float32
BF16 = mybir.dt.bfloat16
RELU = mybir.ActivationFunctionType.Relu
IDENT = mybir.ActivationFunctionType.Identity

CHUNK = 512  # max matmul moving-operand (free dim) size: one PSUM bank fp32

# bias_all column layout: [b1(16) b2(4) eb1(10*8) eb2(10*4) db1(16) db2(8)]
B1_OFF, B2_OFF, EB1_OFF, EB2_OFF, DB1_OFF, DB2_OFF, B_COLS = (
    0, 16, 20, 100, 140, 156, 164)


def _chunks(n, step=CHUNK):
    """Balanced split of n into ceil(n/step) near-equal pieces (all <= step)."""
    nch = -(-n // step)
    base, rem = divmod(n, nch)
    out = []
    s = 0
    for i in range(nch):
        sz = base + (1 if i < rem else 0)
        out.append((s, sz))
        s += sz
    return out


def _enc_chunks(n):
    """First chunk maximal (512) for the k-outer startup path, rest balanced."""
    first = min(CHUNK, n)
    out = [(0, first)]
    if n > first:
        out += [(first + s, l) for s, l in _chunks(n - first)]
    return out


def _dec_chunks(n):
    """Small last chunk so the tail activation+output-DMA is short."""
    if n <= 384:
        return _chunks(n)
    tail = 128
    return _chunks(n - tail) + [(n - tail, tail)]


def _build(n_seg, n_core):
    """Build the SPMD program for per-class-per-core counts n_seg (sum=n_core)."""
    nc = bacc.Bacc()

    xt = nc.dram_tensor("xt", [D_IN, n_core], BF16, kind="ExternalInput")
    w1 = nc.dram_tensor("w1", [D_IN, D_H], BF16, kind="ExternalInput")
    w2 = nc.dram_tensor("w2", [D_H, D_BOT], BF16, kind="ExternalInput")
    ew = nc.dram_tensor("ew", [N_CLS, 128, 8192], BF16, kind="ExternalInput")
    dw1 = nc.dram_tensor("dw1", [D_BOT, D_H], BF16, kind="ExternalInput")
    dw2 = nc.dram_tensor("dw2", [D_H, D_IN], BF16, kind="ExternalInput")
    bias = nc.dram_tensor("bias", [128, B_COLS], F32, kind="ExternalInput")
    out = nc.dram_tensor("out", [D_IN, n_core], F32, kind="ExternalOutput")

    segs = []  # (class e, col start, col len)
    s = 0
    for e in range(N_CLS):
        if n_seg[e] > 0:
            segs.append((e, s, n_seg[e]))
            s += n_seg[e]
    echunks = _enc_chunks(n_core)
    dchunks = _dec_chunks(n_core)

    KT1, MT1 = D_IN // 128, D_H // 128     # enc1: 8, 16
    KT2, MT2 = D_H // 128, D_BOT // 128    # enc2: 16, 4
    KE1, ME1 = D_BOT // 128, D_EXP // 128  # exp1: 4, 8
    KE2, ME2 = D_EXP // 128, D_BOT // 128  # exp2: 8, 4
    KD1, MD1 = D_BOT // 128, D_H // 128    # dec1: 4, 16
    KD2, MD2 = D_H // 128, D_IN // 128     # dec2: 16, 8

    with tile.TileContext(nc) as tc:
        p_const = tc.alloc_tile_pool(name="const", bufs=1)
        p_ps = tc.alloc_tile_pool(name="ps", bufs=8, space="PSUM")

        # ---- PE warmup: ramp the clock during the initial DMA wait ----
        warm_w = p_const.tile([128, 128], BF16, tag="warmw", name="warmw")
        warm_x = p_const.tile([128, CHUNK], BF16, tag="warmx", name="warmx")
        nc.vector.memset(warm_w, 0.0)
        nc.vector.memset(warm_x, 0.0)
        for _ in range(16):
            ps = p_ps.tile([128, CHUNK], F32, tag="ps", name="ps")
            nc.tensor.matmul(ps, warm_w, warm_x, start=True, stop=True)

        # bottleneck activations, SBUF-resident at full width
        p_e2 = tc.alloc_tile_pool(name="e2", bufs=1)
        p_h2 = tc.alloc_tile_pool(name="h2", bufs=1)
        e2_t = [p_e2.tile([128, n_core], BF16, tag=f"e2_{m}", name=f"e2_{m}")
                for m in range(D_BOT // 128)]
        h2_t = [p_h2.tile([128, n_core], BF16, tag=f"h2_{m}", name=f"h2_{m}")
                for m in range(D_BOT // 128)]

        # Expert + decoder-weight arenas are allocated BEFORE the encoder
        # pool: their space never overlaps encoder tiles, so their DMAs carry
        # no false WAR deps and prefetch during the encoder phase.
        EW_BUFS = 4
        ECHUNK = 256
        p_exp = tc.alloc_tile_pool(name="exp", bufs=1)
        e1_ring = [p_exp.tile([128, D_EXP // 128, ECHUNK], BF16, tag=f"e1r_{i}",
                              name=f"e1r_{i}") for i in range(3)]
        # packed per-expert weights: cols [0:4096]=ew1 (k,1024), [4096:8192]=ew2
        ew_ring = [p_exp.tile([128, 8192], BF16, tag=f"ew_{i}", name=f"ew_{i}")
                   for i in range(EW_BUFS)]
        p_decw = tc.alloc_tile_pool(name="decw", bufs=1)
        dw1_tile = p_decw.tile([128, KD1, D_H], BF16, tag="dw1", name="dw1")

        b_t = p_const.tile([128, B_COLS], F32, tag="bias", name="bias")

        # ---------------- encoder ----------------
        p_enc = tc.alloc_tile_pool(name="enc", bufs=1)

        # x chunk 0 as 4 k-pair DMAs so the first matmul waits on 0.26MB only
        c0_start, c0_len = echunks[0]
        xc0 = p_enc.tile([128, KT1, CHUNK], BF16, tag="xc", name="xc", bufs=3)

        def load_xc0_pair(j):
            nc.sync.dma_start(
                out=xc0[:, 2 * j:2 * j + 2, :c0_len],
                in_=xt[j * 256:(j + 1) * 256, c0_start:c0_start + c0_len]
                .rearrange("(a p) n -> p a n", p=128))

        # W1 per-k-plane tiles; k0 split into two column halves
        w1k0 = [p_enc.tile([128, D_H // 2], BF16, tag=f"w1k0h{h}",
                           name=f"w1k0h{h}") for h in range(2)]
        w1k = [None] * KT1

        def load_w1k0_half(h):
            nc.sync.dma_start(
                out=w1k0[h],
                in_=w1[0:128, h * (D_H // 2):(h + 1) * (D_H // 2)])

        def load_w1k(k):
            t = p_enc.tile([128, D_H], BF16, tag=f"w1k{k}", name=f"w1k{k}")
            nc.sync.dma_start(out=t, in_=w1[k * 128:(k + 1) * 128, :])
            w1k[k] = t

        def w1_at(k, m):
            if k == 0:
                return w1k0[m // 8][:, (m % 8) * 128:(m % 8 + 1) * 128]
            return w1k[k][:, m * 128:(m + 1) * 128]

        # startup DMA schedule in need-order
        load_xc0_pair(0)
        load_xc0_pair(1)
        load_w1k0_half(0)
        load_w1k0_half(1)
        load_w1k(1)
        nc.sync.dma_start(out=b_t, in_=bias[:])
        load_w1k(2)
        load_w1k(3)
        load_xc0_pair(2)
        load_w1k(4)
        load_xc0_pair(3)
        load_w1k(5)
        load_w1k(6)
        load_w1k(7)

        def load_w(dram_h, pool, tag, kt, mt_cols, ksplit):
            tiles = []
            per = kt // ksplit
            for i in range(ksplit):
                t = pool.tile([128, per, mt_cols], BF16, tag=f"{tag}{i}",
                              name=f"{tag}{i}")
                nc.sync.dma_start(
                    out=t,
                    in_=dram_h[i * per * 128:(i + 1) * per * 128, :]
                    .rearrange("(a p) n -> p a n", p=128))
                tiles.append(t)
            return lambda k: tiles[k // per][:, k % per, :]

        w2_at = load_w(w2, p_enc, "w2_", KT2, D_BOT, 2)

        def load_xc(c0, cl):
            t = p_enc.tile([128, KT1, CHUNK], BF16, tag="xc", name="xc", bufs=3)
            nc.sync.dma_start(
                out=t[:, :, :cl],
                in_=xt[:, c0:c0 + cl].rearrange("(a p) n -> p a n", p=128))
            return t

        # bias slice helpers (per-partition [128,1] APs into the packed tile)
        b1_c = lambda m: b_t[:, B1_OFF + m:B1_OFF + m + 1]
        b2_c = lambda m: b_t[:, B2_OFF + m:B2_OFF + m + 1]
        eb1_c = lambda e, m: b_t[:, EB1_OFF + e * ME1 + m:EB1_OFF + e * ME1 + m + 1]
        eb2_c = lambda e, m: b_t[:, EB2_OFF + e * ME2 + m:EB2_OFF + e * ME2 + m + 1]
        db1_c = lambda m: b_t[:, DB1_OFF + m:DB1_OFF + m + 1]
        db2_c = lambda m: b_t[:, DB2_OFF + m:DB2_OFF + m + 1]

        # experts are emitted as soon as the encoder chunks covering their
        # column segment are done: their compute absorbs expert-weight DMA
        # latency, and the PE never waits on the weight stream at phase end.
        seg_queue = list(segs)
        exp_counter = [0]
        unit_ctr = [0]
        pend = [None]  # exp2 of each unit is delayed one unit behind its exp1

        def emit_exp1(u):
            e, a, al, slot, ew_t = u
            e1c = e1_ring[slot]
            for m in range(ME1):
                ps = p_ps.tile([128, al], F32, tag="ps", name="ps")
                for k in range(KE1):
                    nc.tensor.matmul(
                        ps,
                        ew_t[:, k * D_EXP + m * 128:k * D_EXP + (m + 1) * 128],
                        h2_t[k][:, a:a + al],
                        start=(k == 0), stop=(k == KE1 - 1))
                # bias+relu on the (idle) vector engine: keeps PSUM
                # evacuation off the scalar engine's critical path
                nc.vector.tensor_scalar(
                    out=e1c[:, m, :al], in0=ps,
                    scalar1=eb1_c(e, m), scalar2=0.0,
                    op0=mybir.AluOpType.add, op1=mybir.AluOpType.max)

        def emit_exp2(u):
            e, a, al, slot, ew_t = u
            e1c = e1_ring[slot]
            for m in range(ME2):
                ps = p_ps.tile([128, al], F32, tag="ps", name="ps")
                for k in range(KE2):
                    nc.tensor.matmul(
                        ps,
                        ew_t[:, 4096 + k * D_BOT + m * 128:
                             4096 + k * D_BOT + (m + 1) * 128],
                        e1c[:, k, :al],
                        start=(k == 0), stop=(k == KE2 - 1))
                nc.scalar.activation(out=e2_t[m][:, a:a + al], in_=ps,
                                     func=RELU, bias=eb2_c(e, m), scale=1.0)

        def emit_expert(e, s0, sl):
            # exp1(unit i) then exp2(unit i-1): exp1's PSUM evacuations (DVE)
            # overlap the next unit's exp1 matmuls instead of stalling the PE
            ei = exp_counter[0]
            exp_counter[0] += 1
            ew_t = ew_ring[ei % EW_BUFS]
            nc.sync.dma_start(out=ew_t, in_=ew[e])
            for c0, cl in _chunks(sl, ECHUNK):
                u = (e, s0 + c0, cl, unit_ctr[0] % 3, ew_t)
                unit_ctr[0] += 1
                emit_exp1(u)
                if pend[0] is not None:
                    emit_exp2(pend[0])
                pend[0] = u

        for ci, (c0, cl) in enumerate(echunks):
            if ci == 0:
                # enc1 k-OUTER over two m-halves of 8 PSUM banks each: the
                # first matmul needs only the first x k-pair + half W1 k0
                h1c = []
                for half in range(2):
                    pss = [p_ps.tile([128, cl], F32, tag="ps", name="ps")
                           for _ in range(8)]
                    for k in range(KT1):
                        for mi in range(8):
                            nc.tensor.matmul(
                                pss[mi], w1_at(k, half * 8 + mi),
                                xc0[:, k, :cl],
                                start=(k == 0), stop=(k == KT1 - 1))
                    for mi in range(8):
                        m = half * 8 + mi
                        h = p_enc.tile([128, CHUNK], BF16, tag="h1c",
                                       name="h1c", bufs=MT1)
                        nc.scalar.activation(out=h[:, :cl], in_=pss[mi],
                                             func=RELU, bias=b1_c(m),
                                             scale=1.0)
                        h1c.append(h)
            else:
                xc = load_xc(c0, cl)
                h1c = []
                for m in range(MT1):
                    ps = p_ps.tile([128, cl], F32, tag="ps", name="ps")
                    for k in range(KT1):
                        nc.tensor.matmul(ps, w1_at(k, m), xc[:, k, :cl],
                                         start=(k == 0), stop=(k == KT1 - 1))
                    h = p_enc.tile([128, CHUNK], BF16, tag="h1c", name="h1c",
                                   bufs=MT1)
                    nc.scalar.activation(out=h[:, :cl], in_=ps, func=RELU,
                                         bias=b1_c(m), scale=1.0)
                    h1c.append(h)
            for m in range(MT2):
                ps = p_ps.tile([128, cl], F32, tag="ps", name="ps")
                for k in range(KT2):
                    nc.tensor.matmul(ps, w2_at(k)[:, m * 128:(m + 1) * 128],
                                     h1c[k][:, :cl],
                                     start=(k == 0), stop=(k == KT2 - 1))
                nc.scalar.activation(out=h2_t[m][:, c0:c0 + cl], func=RELU,
                                     in_=ps, bias=b2_c(m), scale=1.0)
            # run every expert whose segment is fully covered by done chunks
            chunk_end = c0 + cl
            while seg_queue and seg_queue[0][1] + seg_queue[0][2] <= chunk_end:
                e, s0, sl = seg_queue.pop(0)
                emit_expert(e, s0, sl)

        for e, s0, sl in seg_queue:
            emit_expert(e, s0, sl)
        # dec1 weights: sync DMA right behind the last expert weights into the
        # preallocated tile, so the decoder never waits on them
        nc.sync.dma_start(out=dw1_tile,
                          in_=dw1[:].rearrange("(a p) n -> p a n", p=128))
        if pend[0] is not None:
            emit_exp2(pend[0])
            pend[0] = None

        dw1_at = lambda k: dw1_tile[:, k, :]

        p_enc.release()

        # dw2: gpsimd-triggered (its wait on freed encoder space must not
        # block the sync sequencer), streaming during the expert tail.
        p_dec = tc.alloc_tile_pool(name="dec", bufs=1)
        dw2_tiles = []
        for i in range(2):
            t = p_dec.tile([128, KD2 // 2, D_IN], BF16, tag=f"dw2_{i}",
                           name=f"dw2_{i}")
            nc.gpsimd.dma_start(
                out=t,
                in_=dw2[i * 8 * 128:(i + 1) * 8 * 128, :]
                .rearrange("(a p) n -> p a n", p=128))
            dw2_tiles.append(t)
        dw2_at = lambda k: dw2_tiles[k // 8][:, k % 8, :]

        # ---------------- decoder (fused dec1+dec2 per chunk) -----------------
        for c0, cl in dchunks:
            d1c = []
            for m in range(MD1):
                ps = p_ps.tile([128, cl], F32, tag="ps", name="ps")
                for k in range(KD1):
                    nc.tensor.matmul(ps, dw1_at(k)[:, m * 128:(m + 1) * 128],
                                     e2_t[k][:, c0:c0 + cl],
                                     start=(k == 0), stop=(k == KD1 - 1))
                d = p_dec.tile([128, CHUNK], BF16, tag="d1c", name="d1c",
                               bufs=MD1)
                nc.scalar.activation(out=d[:, :cl], in_=ps, func=RELU,
                                     bias=db1_c(m), scale=1.0)
                d1c.append(d)
            for m in range(MD2):
                ps = p_ps.tile([128, cl], F32, tag="ps", name="ps")
                for k in range(KD2):
                    nc.tensor.matmul(ps, dw2_at(k)[:, m * 128:(m + 1) * 128],
                                     d1c[k][:, :cl],
                                     start=(k == 0), stop=(k == KD2 - 1))
                o_t = p_dec.tile([128, CHUNK], F32, tag="o", name="o", bufs=4)
                nc.scalar.activation(out=o_t[:, :cl], in_=ps, func=IDENT,
                                     bias=db2_c(m), scale=1.0)
                nc.sync.dma_start(out=out[m * 128:(m + 1) * 128, c0:c0 + cl],
                                  in_=o_t[:, :cl])

        p_dec.release()
        p_decw.release()
        p_exp.release()
        p_h2.release()
        p_e2.release()
        p_ps.release()
        p_const.release()

    nc.finalize()
    return nc


_CACHE = {}


def _get_nc(n_seg, n_core):
    key = tuple(n_seg)
    if key not in _CACHE:
        _CACHE[key] = _build(n_seg, n_core)
    return _CACHE[key]


def _bf16(a):
    return np.ascontiguousarray(np.asarray(a, np.float32).astype(ml_dtypes.bfloat16))


def _fm(w, kt):
    """[kt*128, n] row-major -> [128, kt, n] feature-major flat [128, kt*n]."""
    a = np.asarray(w, np.float32).astype(ml_dtypes.bfloat16)
    kt128, n = a.shape
    return a.reshape(kt, 128, n).transpose(1, 0, 2).reshape(128, kt * n)


def _pack_bias(b1, b2, Eb1, Eb2, Db1, Db2):
    out = np.zeros((128, B_COLS), np.float32)

    def put(off, vec, mt):
        out[:, off:off + mt] = np.asarray(vec, np.float32).reshape(mt, 128).T

    put(B1_OFF, b1, D_H // 128)
    put(B2_OFF, b2, D_BOT // 128)
    for e in range(N_CLS):
        put(EB1_OFF + e * (D_EXP // 128), Eb1[e], D_EXP // 128)
        put(EB2_OFF + e * (D_BOT // 128), Eb2[e], D_BOT // 128)
    put(DB1_OFF, Db1, D_H // 128)
    put(DB2_OFF, Db2, D_IN // 128)
    return np.ascontiguousarray(out)


def kernel(x, labels, W1, b1, W2, b2, EW1, Eb1, EW2, Eb2, DW1, Db1, DW2, Db2):
    x = np.asarray(x, dtype=np.float32)
    labels_np = np.asarray(labels).astype(np.int64)
    B = x.shape[0]

    counts = np.bincount(labels_np, minlength=N_CLS)
    n_seg = [int(-(-int(c) // N_CORES)) for c in counts]  # ceil(c/8)
    n_core = int(sum(n_seg))

    # assign tokens: class e sorted tokens padded to 8*n_seg[e], row j -> core j
    order = np.argsort(labels_np, kind="stable")
    idx_by_class = np.split(order, np.cumsum(counts)[:-1])
    core_tok = np.full((N_CORES, n_core), -1, dtype=np.int64)
    off = 0
    for e in range(N_CLS):
        ne = n_seg[e]
        if ne == 0:
            continue
        padded = np.full(N_CORES * ne, -1, dtype=np.int64)
        padded[:counts[e]] = idx_by_class[e]
        core_tok[:, off:off + ne] = padded.reshape(N_CORES, ne)
        off += ne

    # packed per-expert weights [N_CLS, 128, 8192]: ew1 feature-major flat
    # [128, 4*1024] then ew2 feature-major flat [128, 8*512]
    ew = np.empty((N_CLS, 128, 8192), ml_dtypes.bfloat16)
    for e in range(N_CLS):
        ew[e, :, :4096] = _fm(EW1[e], D_BOT // 128)
        ew[e, :, 4096:] = _fm(EW2[e], D_EXP // 128)

    weights = {
        "w1": _bf16(W1), "w2": _bf16(W2), "ew": np.ascontiguousarray(ew),
        "dw1": _bf16(DW1), "dw2": _bf16(DW2),
        "bias": _pack_bias(b1, b2, Eb1, Eb2, Db1, Db2),
    }

    x_bf = x.astype(ml_dtypes.bfloat16)
    in_maps = []
    for j in range(N_CORES):
        ids = core_tok[j]
        valid = ids >= 0
        xc = np.zeros((n_core, D_IN), dtype=ml_dtypes.bfloat16)
        xc[valid] = x_bf[ids[valid]]
        im = {"xt": np.ascontiguousarray(xc.T)}
        im.update(weights)
        in_maps.append(im)

    nc = _get_nc(n_seg, n_core)
    res = run_bass_kernel_spmd(nc, in_maps, core_ids=list(range(N_CORES)))

    out = np.empty((B, D_IN), dtype=np.float32)
    for j in range(N_CORES):
        oc = res.results[j]["out"]  # [D_IN, n_core]
        ids = core_tok[j]
        valid = ids >= 0
        out[ids[valid]] = oc.T[valid]
    return out


# revision 9
# speedup vs baseline: 1.0113x; 1.0029x over previous
"""Class-routed autoencoder (moe_routing) Trainium2 kernel.

Strategy:
- The reference computes ALL 10 experts densely then gathers by label; we
  ROUTE instead: sort tokens by class on the host, split every class's tokens
  evenly across the 8 cores (class counts padded up to a multiple of 8 with
  dummy zero tokens), so every core runs an IDENTICAL program (SPMD) on
  N_core = sum_e ceil(c_e/8) tokens laid out as 10 contiguous single-class
  segments. Expert layers slice the right weight block per segment at
  compile time; no gather/scatter on device.
- Everything runs feature-major ([features, tokens]): weights are the
  stationary matmul operand as-is (out = W.T @ x_fm), the batch is the
  moving/free dim, and per-feature bias + ReLU + PSUM->SBUF evacuation fuse
  into one scalar-engine activation op (bias is per-partition).
- Matmuls run in bf16 (fp32 PSUM accumulate, biases added in fp32).
- The kernel is PE-row-bound (~192us of row streaming at 2.4GHz); the
  optimization work is in eliminating non-streaming time:
  * ~16 warmup matmuls on zeroed tiles run during the initial DMA wait so
    the PE p-state ramp (0.65->2.4GHz over ~3us) completes before real data
    arrives.
  * The first encoder chunk (512 cols) runs enc1 k-OUTER over 8 PSUM banks
    per m-half, so the first matmul needs only the first x k-pair (0.26MB)
    + the first half W1 k-slice (0.25MB) instead of x-chunk + 1MB W1 slice.
    W1 is split into 9 DMAs interleaved with the x k-pair DMAs in
    need-order; W2/xc1/xc2 stream behind them.
  * All 6 bias vectors are packed into ONE [128,164] f32 dram tensor (one
    DMA trigger instead of six; the sync sequencer pays ~1.3us per trigger).
  * Each expert's two weight matrices are packed into ONE [128,8192] bf16
    tensor (one trigger per expert), prefetched through a 4-deep ring whose
    slots are allocated below the encoder arena (no false WAR deps).
  * Decoder dw1 is preallocated below the encoder arena and its sync DMA
    triggers right after the last expert weights, so dec1 never waits
    (the baseline had a ~2us PE gap here); dw2 streams via the gpsimd
    sequencer into released encoder space during the expert tail.
  * Decoder chunks are [~450, ~450, 128]: the small last chunk shrinks the
    post-last-matmul activation+output-DMA tail.
- Expert execution is interleaved into the encoder chunk loop: each expert
  runs as soon as the encoder chunks covering its column segment are done,
  so the expert-weight DMA stream hides entirely under encoder compute.
- Host: permute+transpose x, run 8 cores, inverse-permute the output.
"""

import ml_dtypes
import numpy as np

import concourse.bass as bass
import concourse.mybir as mybir
import concourse.tile as tile
from concourse import bacc
from concourse.bass_utils import run_bass_kernel_spmd

N_CORES = 8
N_CLS = 10
D_IN, D_H, D_BOT, D_EXP = 1024, 2048, 512, 1024

F32 = mybir.dt.float32
BF16 = mybir.dt.bfloat16
RELU = mybir.ActivationFunctionType.Relu
IDENT = mybir.ActivationFunctionType.Identity

CHUNK = 512  # max matmul moving-operand (free dim) size: one PSUM bank fp32

# bias_all column layout: [b1(16) b2(4) eb1(10*8) eb2(10*4) db1(16) db2(8)]
B1_OFF, B2_OFF, EB1_OFF, EB2_OFF, DB1_OFF, DB2_OFF, B_COLS = (
    0, 16, 20, 100, 140, 156, 164)


def _chunks(n, step=CHUNK):
    """Balanced split of n into ceil(n/step) near-equal pieces (all <= step)."""
    nch = -(-n // step)
    base, rem = divmod(n, nch)
    out = []
    s = 0
    for i in range(nch):
        sz = base + (1 if i < rem else 0)
        out.append((s, sz))
        s += sz
    return out


def _enc_chunks(n):
    """First chunk maximal (512) for the k-outer startup path, rest balanced."""
    first = min(CHUNK, n)
    out = [(0, first)]
    if n > first:
        out += [(first + s, l) for s, l in _chunks(n - first)]
    return out


def _dec_chunks(n):
    """Small last chunk so the tail activation+output-DMA is short."""
    if n <= 384:
        return _chunks(n)
    tail = 128
    return _chunks(n - tail) + [(n - tail, tail)]


def _build(n_seg, n_core):
    """Build the SPMD program for per-class-per-core counts n_seg (sum=n_core)."""
    nc = bacc.Bacc()

    xt = nc.dram_tensor("xt", [D_IN, n_core], BF16, kind="ExternalInput")
    w1 = nc.dram_tensor("w1", [D_IN, D_H], BF16, kind="ExternalInput")
    w2 = nc.dram_tensor("w2", [D_H, D_BOT], BF16, kind="ExternalInput")
    ew = nc.dram_tensor("ew", [N_CLS, 128, 8192], BF16, kind="ExternalInput")
    dw1 = nc.dram_tensor("dw1", [D_BOT, D_H], BF16, kind="ExternalInput")
    dw2 = nc.dram_tensor("dw2", [D_H, D_IN], BF16, kind="ExternalInput")
    bias = nc.dram_tensor("bias", [128, B_COLS], F32, kind="ExternalInput")
    out = nc.dram_tensor("out", [D_IN, n_core], F32, kind="ExternalOutput")

    segs = []  # (class e, col start, col len)
    s = 0
    for e in range(N_CLS):
        if n_seg[e] > 0:
            segs.append((e, s, n_seg[e]))
            s += n_seg[e]
    echunks = _enc_chunks(n_core)
    dchunks = _dec_chunks(n_core)

    KT1, MT1 = D_IN // 128, D_H // 128     # enc1: 8, 16
    KT2, MT2 = D_H // 128, D_BOT // 128    # enc2: 16, 4
    KE1, ME1 = D_BOT // 128, D_EXP // 128  # exp1: 4, 8
    KE2, ME2 = D_EXP // 128, D_BOT // 128  # exp2: 8, 4
    KD1, MD1 = D_BOT // 128, D_H // 128    # dec1: 4, 16
    KD2, MD2 = D_H // 128, D_IN // 128     # dec2: 16, 8

    with tile.TileContext(nc) as tc:
        p_const = tc.alloc_tile_pool(name="const", bufs=1)
        p_ps = tc.alloc_tile_pool(name="ps", bufs=8, space="PSUM")

        warm_w = p_const.tile([128, 128], BF16, tag="warmw", name="warmw")

        # bottleneck activations, SBUF-resident at full width
        p_e2 = tc.alloc_tile_pool(name="e2", bufs=1)
        p_h2 = tc.alloc_tile_pool(name="h2", bufs=1)
        e2_t = [p_e2.tile([128, n_core], BF16, tag=f"e2_{m}", name=f"e2_{m}")
                for m in range(D_BOT // 128)]
        h2_t = [p_h2.tile([128, n_core], BF16, tag=f"h2_{m}", name=f"h2_{m}")
                for m in range(D_BOT // 128)]

        # Expert + decoder-weight arenas are allocated BEFORE the encoder
        # pool: their space never overlaps encoder tiles, so their DMAs carry
        # no false WAR deps and prefetch during the encoder phase.
        EW_BUFS = 4
        ECHUNK = 256
        p_exp = tc.alloc_tile_pool(name="exp", bufs=1)
        e1_ring = [p_exp.tile([128, D_EXP // 128, ECHUNK], BF16, tag=f"e1r_{i}",
                              name=f"e1r_{i}") for i in range(3)]
        # packed per-expert weights: cols [0:4096]=ew1 (k,1024), [4096:8192]=ew2
        ew_ring = [p_exp.tile([128, 8192], BF16, tag=f"ew_{i}", name=f"ew_{i}")
                   for i in range(EW_BUFS)]
        p_decw = tc.alloc_tile_pool(name="decw", bufs=1)
        dw1_tile = p_decw.tile([128, KD1, D_H], BF16, tag="dw1", name="dw1")

        b_t = p_const.tile([128, B_COLS], F32, tag="bias", name="bias")

        # ---------------- encoder ----------------
        p_enc = tc.alloc_tile_pool(name="enc", bufs=1)

        # x chunk 0 as 4 k-pair DMAs so the first matmul waits on 0.26MB only
        c0_start, c0_len = echunks[0]
        xc0 = p_enc.tile([128, KT1, CHUNK], BF16, tag="xc", name="xc", bufs=3)

        def load_xc0_pair(j):
            nc.sync.dma_start(
                out=xc0[:, 2 * j:2 * j + 2, :c0_len],
                in_=xt[j * 256:(j + 1) * 256, c0_start:c0_start + c0_len]
                .rearrange("(a p) n -> p a n", p=128))

        # W1 per-k-plane tiles; k0 split into two column halves
        w1k0 = [p_enc.tile([128, D_H // 2], BF16, tag=f"w1k0h{h}",
                           name=f"w1k0h{h}") for h in range(2)]
        w1k = [None] * KT1

        def load_w1k0_half(h):
            nc.sync.dma_start(
                out=w1k0[h],
                in_=w1[0:128, h * (D_H // 2):(h + 1) * (D_H // 2)])

        def load_w1k(k):
            t = p_enc.tile([128, D_H], BF16, tag=f"w1k{k}", name=f"w1k{k}")
            nc.sync.dma_start(out=t, in_=w1[k * 128:(k + 1) * 128, :])
            w1k[k] = t

        def w1_at(k, m):
            if k == 0:
                return w1k0[m // 8][:, (m % 8) * 128:(m % 8 + 1) * 128]
            return w1k[k][:, m * 128:(m + 1) * 128]

        # startup DMA schedule in need-order (emitted before any other
        # engine work so the sync sequencer triggers them immediately)
        load_xc0_pair(0)
        load_xc0_pair(1)
        load_w1k0_half(0)
        load_w1k0_half(1)
        load_w1k(1)
        nc.sync.dma_start(out=b_t, in_=bias[:])
        load_w1k(2)
        load_w1k(3)
        load_xc0_pair(2)
        load_w1k(4)
        load_xc0_pair(3)
        load_w1k(5)
        load_w1k(6)
        load_w1k(7)

        # ---- PE warmup: ramp the clock during the initial DMA wait ----
        # ~56 tiny matmuls (~4.7us at the ramping clock) keep the PE busy
        # until the first real operands land, so the p-state reaches 2.4GHz
        # before real work starts instead of ramping through it.
        nc.vector.memset(warm_w, 0.0)
        for _ in range(56):
            ps = p_ps.tile([128, 128], F32, tag="ps", name="ps")
            nc.tensor.matmul(ps, warm_w, warm_w, start=True, stop=True)

        def load_w(dram_h, pool, tag, kt, mt_cols, ksplit):
            tiles = []
            per = kt // ksplit
            for i in range(ksplit):
                t = pool.tile([128, per, mt_cols], BF16, tag=f"{tag}{i}",
                              name=f"{tag}{i}")
                nc.sync.dma_start(
                    out=t,
                    in_=dram_h[i * per * 128:(i + 1) * per * 128, :]
                    .rearrange("(a p) n -> p a n", p=128))
                tiles.append(t)
            return lambda k: tiles[k // per][:, k % per, :]

        w2_at = load_w(w2, p_enc, "w2_", KT2, D_BOT, 2)

        def load_xc(c0, cl):
            t = p_enc.tile([128, KT1, CHUNK], BF16, tag="xc", name="xc", bufs=3)
            nc.sync.dma_start(
                out=t[:, :, :cl],
                in_=xt[:, c0:c0 + cl].rearrange("(a p) n -> p a n", p=128))
            return t

        # bias slice helpers (per-partition [128,1] APs into the packed tile)
        b1_c = lambda m: b_t[:, B1_OFF + m:B1_OFF + m + 1]
        b2_c = lambda m: b_t[:, B2_OFF + m:B2_OFF + m + 1]
        eb1_c = lambda e, m: b_t[:, EB1_OFF + e * ME1 + m:EB1_OFF + e * ME1 + m + 1]
        eb2_c = lambda e, m: b_t[:, EB2_OFF + e * ME2 + m:EB2_OFF + e * ME2 + m + 1]
        db1_c = lambda m: b_t[:, DB1_OFF + m:DB1_OFF + m + 1]
        db2_c = lambda m: b_t[:, DB2_OFF + m:DB2_OFF + m + 1]

        # experts are emitted as soon as the encoder chunks covering their
        # column segment are done: their compute absorbs expert-weight DMA
        # latency, and the PE never waits on the weight stream at phase end.
        seg_queue = list(segs)
        exp_counter = [0]
        unit_ctr = [0]
        pend = [None]  # exp2 of each unit is delayed one unit behind its exp1

        def emit_exp1(u):
            e, a, al, slot, ew_t = u
            e1c = e1_ring[slot]
            for m in range(ME1):
                ps = p_ps.tile([128, al], F32, tag="ps", name="ps")
                for k in range(KE1):
                    nc.tensor.matmul(
                        ps,
                        ew_t[:, k * D_EXP + m * 128:k * D_EXP + (m + 1) * 128],
                        h2_t[k][:, a:a + al],
                        start=(k == 0), stop=(k == KE1 - 1))
                # bias+relu on the (idle) vector engine: keeps PSUM
                # evacuation off the scalar engine's critical path
                nc.vector.tensor_scalar(
                    out=e1c[:, m, :al], in0=ps,
                    scalar1=eb1_c(e, m), scalar2=0.0,
                    op0=mybir.AluOpType.add, op1=mybir.AluOpType.max)

        def emit_exp2(u):
            e, a, al, slot, ew_t = u
            e1c = e1_ring[slot]
            for m in range(ME2):
                ps = p_ps.tile([128, al], F32, tag="ps", name="ps")
                for k in range(KE2):
                    nc.tensor.matmul(
                        ps,
                        ew_t[:, 4096 + k * D_BOT + m * 128:
                             4096 + k * D_BOT + (m + 1) * 128],
                        e1c[:, k, :al],
                        start=(k == 0), stop=(k == KE2 - 1))
                nc.scalar.activation(out=e2_t[m][:, a:a + al], in_=ps,
                                     func=RELU, bias=eb2_c(e, m), scale=1.0)

        def emit_expert(e, s0, sl):
            # exp1(unit i) then exp2(unit i-1): exp1's PSUM evacuations (DVE)
            # overlap the next unit's exp1 matmuls instead of stalling the PE
            ei = exp_counter[0]
            exp_counter[0] += 1
            ew_t = ew_ring[ei % EW_BUFS]
            nc.sync.dma_start(out=ew_t, in_=ew[e])
            for c0, cl in _chunks(sl, ECHUNK):
                u = (e, s0 + c0, cl, unit_ctr[0] % 3, ew_t)
                unit_ctr[0] += 1
                emit_exp1(u)
                if pend[0] is not None:
                    emit_exp2(pend[0])
                pend[0] = u

        xc_next = [None]
        for ci, (c0, cl) in enumerate(echunks):
            xc = xc_next[0]
            if ci == 0:
                # enc1 k-OUTER over two m-halves of 8 PSUM banks each: the
                # first matmul needs only the first x k-pair + half W1 k0
                h1c = []
                for half in range(2):
                    pss = [p_ps.tile([128, cl], F32, tag="ps", name="ps")
                           for _ in range(8)]
                    for k in range(KT1):
                        for mi in range(8):
                            nc.tensor.matmul(
                                pss[mi], w1_at(k, half * 8 + mi),
                                xc0[:, k, :cl],
                                start=(k == 0), stop=(k == KT1 - 1))
                    for mi in range(8):
                        m = half * 8 + mi
                        h = p_enc.tile([128, CHUNK], BF16, tag="h1c",
                                       name="h1c", bufs=MT1)
                        nc.scalar.activation(out=h[:, :cl], in_=pss[mi],
                                             func=RELU, bias=b1_c(m),
                                             scale=1.0)
                        h1c.append(h)
            else:
                h1c = []
                for m in range(MT1):
                    ps = p_ps.tile([128, cl], F32, tag="ps", name="ps")
                    for k in range(KT1):
                        nc.tensor.matmul(ps, w1_at(k, m), xc[:, k, :cl],
                                         start=(k == 0), stop=(k == KT1 - 1))
                    h = p_enc.tile([128, CHUNK], BF16, tag="h1c", name="h1c",
                                   bufs=MT1)
                    nc.scalar.activation(out=h[:, :cl], in_=ps, func=RELU,
                                         bias=b1_c(m), scale=1.0)
                    h1c.append(h)
            for m in range(MT2):
                ps = p_ps.tile([128, cl], F32, tag="ps", name="ps")
                for k in range(KT2):
                    nc.tensor.matmul(ps, w2_at(k)[:, m * 128:(m + 1) * 128],
                                     h1c[k][:, :cl],
                                     start=(k == 0), stop=(k == KT2 - 1))
                nc.scalar.activation(out=h2_t[m][:, c0:c0 + cl], func=RELU,
                                     in_=ps, bias=b2_c(m), scale=1.0)
            # prefetch the next chunk's x ahead of the expert-weight triggers
            if ci + 1 < len(echunks):
                xc_next[0] = load_xc(*echunks[ci + 1])
            # run every expert whose segment is fully covered by done chunks
            chunk_end = c0 + cl
            while seg_queue and seg_queue[0][1] + seg_queue[0][2] <= chunk_end:
                e, s0, sl = seg_queue.pop(0)
                emit_expert(e, s0, sl)

        for e, s0, sl in seg_queue:
            emit_expert(e, s0, sl)
        # dec1 weights: sync DMA right behind the last expert weights into the
        # preallocated tile, so the decoder never waits on them
        nc.sync.dma_start(out=dw1_tile,
                          in_=dw1[:].rearrange("(a p) n -> p a n", p=128))
        if pend[0] is not None:
            emit_exp2(pend[0])
            pend[0] = None

        dw1_at = lambda k: dw1_tile[:, k, :]

        p_enc.release()

        # dw2: gpsimd-triggered (its wait on freed encoder space must not
        # block the sync sequencer), streaming during the expert tail.
        p_dec = tc.alloc_tile_pool(name="dec", bufs=1)
        dw2_tiles = []
        for i in range(2):
            t = p_dec.tile([128, KD2 // 2, D_IN], BF16, tag=f"dw2_{i}",
                           name=f"dw2_{i}")
            nc.gpsimd.dma_start(
                out=t,
                in_=dw2[i * 8 * 128:(i + 1) * 8 * 128, :]
                .rearrange("(a p) n -> p a n", p=128))
            dw2_tiles.append(t)
        dw2_at = lambda k: dw2_tiles[k // 8][:, k % 8, :]

        # ---------------- decoder (fused dec1+dec2 per chunk) -----------------
        for c0, cl in dchunks:
            d1c = []
            for m in range(MD1):
                ps = p_ps.tile([128, cl], F32, tag="ps", name="ps")
                for k in range(KD1):
                    nc.tensor.matmul(ps, dw1_at(k)[:, m * 128:(m + 1) * 128],
                                     e2_t[k][:, c0:c0 + cl],
                                     start=(k == 0), stop=(k == KD1 - 1))
                d = p_dec.tile([128, CHUNK], BF16, tag="d1c", name="d1c",
                               bufs=MD1)
                nc.scalar.activation(out=d[:, :cl], in_=ps, func=RELU,
                                     bias=db1_c(m), scale=1.0)
                d1c.append(d)
            for m in range(MD2):
                ps = p_ps.tile([128, cl], F32, tag="ps", name="ps")
                for k in range(KD2):
                    nc.tensor.matmul(ps, dw2_at(k)[:, m * 128:(m + 1) * 128],
                                     d1c[k][:, :cl],
                                     start=(k == 0), stop=(k == KD2 - 1))
                o_t = p_dec.tile([128, CHUNK], F32, tag="o", name="o", bufs=8)
                nc.scalar.activation(out=o_t[:, :cl], in_=ps, func=IDENT,
                                     bias=db2_c(m), scale=1.0)
                nc.sync.dma_start(out=out[m * 128:(m + 1) * 128, c0:c0 + cl],
                                  in_=o_t[:, :cl])

        p_dec.release()
        p_decw.release()
        p_exp.release()
        p_h2.release()
        p_e2.release()
        p_ps.release()
        p_const.release()

    nc.finalize()
    return nc


_CACHE = {}


def _get_nc(n_seg, n_core):
    key = tuple(n_seg)
    if key not in _CACHE:
        _CACHE[key] = _build(n_seg, n_core)
    return _CACHE[key]


def _bf16(a):
    return np.ascontiguousarray(np.asarray(a, np.float32).astype(ml_dtypes.bfloat16))


def _fm(w, kt):
    """[kt*128, n] row-major -> [128, kt, n] feature-major flat [128, kt*n]."""
    a = np.asarray(w, np.float32).astype(ml_dtypes.bfloat16)
    kt128, n = a.shape
    return a.reshape(kt, 128, n).transpose(1, 0, 2).reshape(128, kt * n)


def _pack_bias(b1, b2, Eb1, Eb2, Db1, Db2):
    out = np.zeros((128, B_COLS), np.float32)

    def put(off, vec, mt):
        out[:, off:off + mt] = np.asarray(vec, np.float32).reshape(mt, 128).T

    put(B1_OFF, b1, D_H // 128)
    put(B2_OFF, b2, D_BOT // 128)
    for e in range(N_CLS):
        put(EB1_OFF + e * (D_EXP // 128), Eb1[e], D_EXP // 128)
        put(EB2_OFF + e * (D_BOT // 128), Eb2[e], D_BOT // 128)
    put(DB1_OFF, Db1, D_H // 128)
    put(DB2_OFF, Db2, D_IN // 128)
    return np.ascontiguousarray(out)


def kernel(x, labels, W1, b1, W2, b2, EW1, Eb1, EW2, Eb2, DW1, Db1, DW2, Db2):
    x = np.asarray(x, dtype=np.float32)
    labels_np = np.asarray(labels).astype(np.int64)
    B = x.shape[0]

    counts = np.bincount(labels_np, minlength=N_CLS)
    n_seg = [int(-(-int(c) // N_CORES)) for c in counts]  # ceil(c/8)
    n_core = int(sum(n_seg))

    # assign tokens: class e sorted tokens padded to 8*n_seg[e], row j -> core j
    order = np.argsort(labels_np, kind="stable")
    idx_by_class = np.split(order, np.cumsum(counts)[:-1])
    core_tok = np.full((N_CORES, n_core), -1, dtype=np.int64)
    off = 0
    for e in range(N_CLS):
        ne = n_seg[e]
        if ne == 0:
            continue
        padded = np.full(N_CORES * ne, -1, dtype=np.int64)
        padded[:counts[e]] = idx_by_class[e]
        core_tok[:, off:off + ne] = padded.reshape(N_CORES, ne)
        off += ne

    # packed per-expert weights [N_CLS, 128, 8192]: ew1 feature-major flat
    # [128, 4*1024] then ew2 feature-major flat [128, 8*512]
    ew = np.empty((N_CLS, 128, 8192), ml_dtypes.bfloat16)
    for e in range(N_CLS):
        ew[e, :, :4096] = _fm(EW1[e], D_BOT // 128)
        ew[e, :, 4096:] = _fm(EW2[e], D_EXP // 128)

    weights = {
        "w1": _bf16(W1), "w2": _bf16(W2), "ew": np.ascontiguousarray(ew),
        "dw1": _bf16(DW1), "dw2": _bf16(DW2),
        "bias": _pack_bias(b1, b2, Eb1, Eb2, Db1, Db2),
    }

    x_bf = x.astype(ml_dtypes.bfloat16)
    in_maps = []
    for j in range(N_CORES):
        ids = core_tok[j]
        valid = ids >= 0
        xc = np.zeros((n_core, D_IN), dtype=ml_dtypes.bfloat16)
        xc[valid] = x_bf[ids[valid]]
        im = {"xt": np.ascontiguousarray(xc.T)}
        im.update(weights)
        in_maps.append(im)

    nc = _get_nc(n_seg, n_core)
    res = run_bass_kernel_spmd(nc, in_maps, core_ids=list(range(N_CORES)))

    out = np.empty((B, D_IN), dtype=np.float32)
    for j in range(N_CORES):
        oc = res.results[j]["out"]  # [D_IN, n_core]
        ids = core_tok[j]
        valid = ids >= 0
        out[ids[valid]] = oc.T[valid]
    return out


# revision 11
# speedup vs baseline: 1.0167x; 1.0053x over previous
"""Class-routed autoencoder (moe_routing) Trainium2 kernel.

Strategy:
- The reference computes ALL 10 experts densely then gathers by label; we
  ROUTE instead: sort tokens by class on the host, split every class's tokens
  evenly across the 8 cores (class counts padded up to a multiple of 8 with
  dummy zero tokens), so every core runs an IDENTICAL program (SPMD) on
  N_core = sum_e ceil(c_e/8) tokens laid out as 10 contiguous single-class
  segments. Expert layers slice the right weight block per segment at
  compile time; no gather/scatter on device.
- Everything runs feature-major ([features, tokens]): weights are the
  stationary matmul operand as-is (out = W.T @ x_fm), the batch is the
  moving/free dim, and per-feature bias + ReLU + PSUM->SBUF evacuation fuse
  into one scalar-engine activation op (bias is per-partition).
- Matmuls run in bf16 (fp32 PSUM accumulate, biases added in fp32).
- The kernel is PE-row-bound (~192us of row streaming at 2.4GHz); the
  optimization work is in eliminating non-streaming time:
  * ~16 warmup matmuls on zeroed tiles run during the initial DMA wait so
    the PE p-state ramp (0.65->2.4GHz over ~3us) completes before real data
    arrives.
  * The first encoder chunk (512 cols) runs enc1 k-OUTER over 8 PSUM banks
    per m-half, so the first matmul needs only the first x k-pair (0.26MB)
    + the first half W1 k-slice (0.25MB) instead of x-chunk + 1MB W1 slice.
    W1 is split into 9 DMAs interleaved with the x k-pair DMAs in
    need-order; W2/xc1/xc2 stream behind them.
  * All 6 bias vectors are packed into ONE [128,164] f32 dram tensor (one
    DMA trigger instead of six; the sync sequencer pays ~1.3us per trigger).
  * Each expert's two weight matrices are packed into ONE [128,8192] bf16
    tensor (one trigger per expert), prefetched through a 4-deep ring whose
    slots are allocated below the encoder arena (no false WAR deps).
  * Decoder dw1 is preallocated below the encoder arena and its sync DMA
    triggers right after the last expert weights, so dec1 never waits
    (the baseline had a ~2us PE gap here); dw2 streams via the gpsimd
    sequencer into released encoder space during the expert tail.
  * Decoder chunks are [~450, ~450, 128]: the small last chunk shrinks the
    post-last-matmul activation+output-DMA tail.
- Expert execution is interleaved into the encoder chunk loop: each expert
  runs as soon as the encoder chunks covering its column segment are done,
  so the expert-weight DMA stream hides entirely under encoder compute.
- Host: permute+transpose x, run 8 cores, inverse-permute the output.
"""

import ml_dtypes
import numpy as np

import concourse.bass as bass
import concourse.mybir as mybir
import concourse.tile as tile
from concourse import bacc
from concourse.bass_utils import run_bass_kernel_spmd

N_CORES = 8
N_CLS = 10
D_IN, D_H, D_BOT, D_EXP = 1024, 2048, 512, 1024

F32 = mybir.dt.float32
BF16 = mybir.dt.bfloat16
RELU = mybir.ActivationFunctionType.Relu
IDENT = mybir.ActivationFunctionType.Identity

CHUNK = 512  # max matmul moving-operand (free dim) size: one PSUM bank fp32

# bias_all column layout: [b1(16) b2(4) eb1(10*8) eb2(10*4) db1(16) db2(8)]
B1_OFF, B2_OFF, EB1_OFF, EB2_OFF, DB1_OFF, DB2_OFF, B_COLS = (
    0, 16, 20, 100, 140, 156, 164)


def _chunks(n, step=CHUNK):
    """Balanced split of n into ceil(n/step) near-equal pieces (all <= step)."""
    nch = -(-n // step)
    base, rem = divmod(n, nch)
    out = []
    s = 0
    for i in range(nch):
        sz = base + (1 if i < rem else 0)
        out.append((s, sz))
        s += sz
    return out


def _enc_chunks(n):
    """First chunk maximal (512) for the k-outer startup path, rest balanced."""
    first = min(CHUNK, n)
    out = [(0, first)]
    if n > first:
        out += [(first + s, l) for s, l in _chunks(n - first)]
    return out


def _dec_chunks(n):
    """Small last chunk so the tail activation+output-DMA is short."""
    if n <= 384:
        return _chunks(n)
    tail = 128
    return _chunks(n - tail) + [(n - tail, tail)]


def _build(n_seg, n_core):
    """Build the SPMD program for per-class-per-core counts n_seg (sum=n_core)."""
    nc = bacc.Bacc()

    xt = nc.dram_tensor("xt", [D_IN, n_core], BF16, kind="ExternalInput")
    w1 = nc.dram_tensor("w1", [D_IN, D_H], BF16, kind="ExternalInput")
    w2 = nc.dram_tensor("w2", [D_H, D_BOT], BF16, kind="ExternalInput")
    ew = nc.dram_tensor("ew", [N_CLS, 128, 8192], BF16, kind="ExternalInput")
    dw1 = nc.dram_tensor("dw1", [D_BOT, D_H], BF16, kind="ExternalInput")
    dw2 = nc.dram_tensor("dw2", [D_H, D_IN], BF16, kind="ExternalInput")
    bias = nc.dram_tensor("bias", [128, B_COLS], F32, kind="ExternalInput")
    out = nc.dram_tensor("out", [D_IN, n_core], F32, kind="ExternalOutput")

    segs = []  # (class e, col start, col len)
    s = 0
    for e in range(N_CLS):
        if n_seg[e] > 0:
            segs.append((e, s, n_seg[e]))
            s += n_seg[e]
    echunks = _enc_chunks(n_core)
    dchunks = _dec_chunks(n_core)

    KT1, MT1 = D_IN // 128, D_H // 128     # enc1: 8, 16
    KT2, MT2 = D_H // 128, D_BOT // 128    # enc2: 16, 4
    KE1, ME1 = D_BOT // 128, D_EXP // 128  # exp1: 4, 8
    KE2, ME2 = D_EXP // 128, D_BOT // 128  # exp2: 8, 4
    KD1, MD1 = D_BOT // 128, D_H // 128    # dec1: 4, 16
    KD2, MD2 = D_H // 128, D_IN // 128     # dec2: 16, 8

    with tile.TileContext(nc) as tc:
        p_const = tc.alloc_tile_pool(name="const", bufs=1)
        p_ps = tc.alloc_tile_pool(name="ps", bufs=8, space="PSUM")

        warm_w = p_const.tile([128, 128], BF16, tag="warmw", name="warmw")

        # bottleneck activations, SBUF-resident at full width
        p_e2 = tc.alloc_tile_pool(name="e2", bufs=1)
        p_h2 = tc.alloc_tile_pool(name="h2", bufs=1)
        e2_t = [p_e2.tile([128, n_core], BF16, tag=f"e2_{m}", name=f"e2_{m}")
                for m in range(D_BOT // 128)]
        h2_t = [p_h2.tile([128, n_core], BF16, tag=f"h2_{m}", name=f"h2_{m}")
                for m in range(D_BOT // 128)]

        # Expert + decoder-weight arenas are allocated BEFORE the encoder
        # pool: their space never overlaps encoder tiles, so their DMAs carry
        # no false WAR deps and prefetch during the encoder phase.
        EW_BUFS = 4
        ECHUNK = 256
        p_exp = tc.alloc_tile_pool(name="exp", bufs=1)
        e1_ring = [p_exp.tile([128, D_EXP // 128, ECHUNK], BF16, tag=f"e1r_{i}",
                              name=f"e1r_{i}") for i in range(3)]
        # packed per-expert weights: cols [0:4096]=ew1 (k,1024), [4096:8192]=ew2
        ew_ring = [p_exp.tile([128, 8192], BF16, tag=f"ew_{i}", name=f"ew_{i}")
                   for i in range(EW_BUFS)]
        p_decw = tc.alloc_tile_pool(name="decw", bufs=1)
        dw1_tile = p_decw.tile([128, KD1, D_H], BF16, tag="dw1", name="dw1")

        b_t = p_const.tile([128, B_COLS], F32, tag="bias", name="bias")

        # ---------------- encoder ----------------
        p_enc = tc.alloc_tile_pool(name="enc", bufs=1)

        # x chunk 0 as 4 k-pair DMAs so the first matmul waits on 0.26MB only
        c0_start, c0_len = echunks[0]
        xc0 = p_enc.tile([128, KT1, CHUNK], BF16, tag="xc", name="xc", bufs=3)

        def load_xc0_pair(j, eng):
            eng.dma_start(
                out=xc0[:, 2 * j:2 * j + 2, :c0_len],
                in_=xt[j * 256:(j + 1) * 256, c0_start:c0_start + c0_len]
                .rearrange("(a p) n -> p a n", p=128))

        # W1 per-k-plane tiles; k0 split into two column halves
        w1k0 = [p_enc.tile([128, D_H // 2], BF16, tag=f"w1k0h{h}",
                           name=f"w1k0h{h}") for h in range(2)]
        w1k = [None] * KT1

        def load_w1k0_half(h, eng):
            eng.dma_start(
                out=w1k0[h],
                in_=w1[0:128, h * (D_H // 2):(h + 1) * (D_H // 2)])

        def load_w1k(k):
            t = p_enc.tile([128, D_H], BF16, tag=f"w1k{k}", name=f"w1k{k}")
            nc.sync.dma_start(out=t, in_=w1[k * 128:(k + 1) * 128, :])
            w1k[k] = t

        def w1_at(k, m):
            if k == 0:
                return w1k0[m // 8][:, (m % 8) * 128:(m % 8 + 1) * 128]
            return w1k[k][:, m * 128:(m + 1) * 128]

        # Startup DMA schedule in need-order. The sync sequencer's prologue
        # runs ~2.5us longer than gpsimd/scalar/vector's and every trigger
        # costs ~0.7us of sequencer time, so the first (critical-path) DMAs
        # are issued from the other engines' queues in parallel.
        load_xc0_pair(0, nc.gpsimd)     # planes 0-1, covers enc1 k=0,1
        load_w1k0_half(0, nc.scalar)    # W1 k0, m0-7 cols
        load_w1k0_half(1, nc.gpsimd)    # W1 k0, m8-15 cols
        load_xc0_pair(1, nc.sync)       # planes 2-3
        load_w1k(1)
        load_w1k(2)
        load_xc0_pair(2, nc.sync)
        load_w1k(3)
        nc.sync.dma_start(out=b_t, in_=bias[:])
        load_xc0_pair(3, nc.sync)
        load_w1k(4)
        load_w1k(5)
        load_w1k(6)
        load_w1k(7)

        # ---- PE warmup: ramp the clock during the initial DMA wait ----
        # Tiny matmuls keep the PE busy until the first real operands land,
        # so the p-state reaches 2.4GHz before real work starts instead of
        # ramping through it.
        nc.vector.memset(warm_w, 0.0)
        for _ in range(44):
            ps = p_ps.tile([128, 128], F32, tag="ps", name="ps")
            nc.tensor.matmul(ps, warm_w, warm_w, start=True, stop=True)

        def load_w(dram_h, pool, tag, kt, mt_cols, ksplit):
            tiles = []
            per = kt // ksplit
            for i in range(ksplit):
                t = pool.tile([128, per, mt_cols], BF16, tag=f"{tag}{i}",
                              name=f"{tag}{i}")
                nc.sync.dma_start(
                    out=t,
                    in_=dram_h[i * per * 128:(i + 1) * per * 128, :]
                    .rearrange("(a p) n -> p a n", p=128))
                tiles.append(t)
            return lambda k: tiles[k // per][:, k % per, :]

        w2_at = load_w(w2, p_enc, "w2_", KT2, D_BOT, 2)

        def load_xc(c0, cl):
            t = p_enc.tile([128, KT1, CHUNK], BF16, tag="xc", name="xc", bufs=3)
            nc.sync.dma_start(
                out=t[:, :, :cl],
                in_=xt[:, c0:c0 + cl].rearrange("(a p) n -> p a n", p=128))
            return t

        # bias slice helpers (per-partition [128,1] APs into the packed tile)
        b1_c = lambda m: b_t[:, B1_OFF + m:B1_OFF + m + 1]
        b2_c = lambda m: b_t[:, B2_OFF + m:B2_OFF + m + 1]
        eb1_c = lambda e, m: b_t[:, EB1_OFF + e * ME1 + m:EB1_OFF + e * ME1 + m + 1]
        eb2_c = lambda e, m: b_t[:, EB2_OFF + e * ME2 + m:EB2_OFF + e * ME2 + m + 1]
        db1_c = lambda m: b_t[:, DB1_OFF + m:DB1_OFF + m + 1]
        db2_c = lambda m: b_t[:, DB2_OFF + m:DB2_OFF + m + 1]

        # experts are emitted as soon as the encoder chunks covering their
        # column segment are done: their compute absorbs expert-weight DMA
        # latency, and the PE never waits on the weight stream at phase end.
        seg_queue = list(segs)
        exp_counter = [0]
        unit_ctr = [0]
        pend = [None]  # exp2 of each unit is delayed one unit behind its exp1

        def emit_exp1(u):
            e, a, al, slot, ew_t = u
            e1c = e1_ring[slot]
            for m in range(ME1):
                ps = p_ps.tile([128, al], F32, tag="ps", name="ps")
                for k in range(KE1):
                    nc.tensor.matmul(
                        ps,
                        ew_t[:, k * D_EXP + m * 128:k * D_EXP + (m + 1) * 128],
                        h2_t[k][:, a:a + al],
                        start=(k == 0), stop=(k == KE1 - 1))
                # bias+relu on the (idle) vector engine: keeps PSUM
                # evacuation off the scalar engine's critical path
                nc.vector.tensor_scalar(
                    out=e1c[:, m, :al], in0=ps,
                    scalar1=eb1_c(e, m), scalar2=0.0,
                    op0=mybir.AluOpType.add, op1=mybir.AluOpType.max)

        def emit_exp2(u):
            e, a, al, slot, ew_t = u
            e1c = e1_ring[slot]
            for m in range(ME2):
                ps = p_ps.tile([128, al], F32, tag="ps", name="ps")
                for k in range(KE2):
                    nc.tensor.matmul(
                        ps,
                        ew_t[:, 4096 + k * D_BOT + m * 128:
                             4096 + k * D_BOT + (m + 1) * 128],
                        e1c[:, k, :al],
                        start=(k == 0), stop=(k == KE2 - 1))
                nc.scalar.activation(out=e2_t[m][:, a:a + al], in_=ps,
                                     func=RELU, bias=eb2_c(e, m), scale=1.0)

        def emit_expert(e, s0, sl):
            # exp1(unit i) then exp2(unit i-1): exp1's PSUM evacuations (DVE)
            # overlap the next unit's exp1 matmuls instead of stalling the PE
            ei = exp_counter[0]
            exp_counter[0] += 1
            ew_t = ew_ring[ei % EW_BUFS]
            nc.sync.dma_start(out=ew_t, in_=ew[e])
            for c0, cl in _chunks(sl, ECHUNK):
                u = (e, s0 + c0, cl, unit_ctr[0] % 3, ew_t)
                unit_ctr[0] += 1
                emit_exp1(u)
                if pend[0] is not None:
                    emit_exp2(pend[0])
                pend[0] = u

        xc_next = [None]
        for ci, (c0, cl) in enumerate(echunks):
            xc = xc_next[0]
            if ci == 0:
                # enc1 k-OUTER over two m-halves of 8 PSUM banks each: the
                # first matmul needs only the first x k-pair + half W1 k0
                h1c = []
                for half in range(2):
                    pss = [p_ps.tile([128, cl], F32, tag="ps", name="ps")
                           for _ in range(8)]
                    for k in range(KT1):
                        for mi in range(8):
                            nc.tensor.matmul(
                                pss[mi], w1_at(k, half * 8 + mi),
                                xc0[:, k, :cl],
                                start=(k == 0), stop=(k == KT1 - 1))
                    for mi in range(8):
                        m = half * 8 + mi
                        h = p_enc.tile([128, CHUNK], BF16, tag="h1c",
                                       name="h1c", bufs=MT1)
                        nc.scalar.activation(out=h[:, :cl], in_=pss[mi],
                                             func=RELU, bias=b1_c(m),
                                             scale=1.0)
                        h1c.append(h)
            else:
                h1c = []
                for m in range(MT1):
                    ps = p_ps.tile([128, cl], F32, tag="ps", name="ps")
                    for k in range(KT1):
                        nc.tensor.matmul(ps, w1_at(k, m), xc[:, k, :cl],
                                         start=(k == 0), stop=(k == KT1 - 1))
                    h = p_enc.tile([128, CHUNK], BF16, tag="h1c", name="h1c",
                                   bufs=MT1)
                    nc.scalar.activation(out=h[:, :cl], in_=ps, func=RELU,
                                         bias=b1_c(m), scale=1.0)
                    h1c.append(h)
            for m in range(MT2):
                ps = p_ps.tile([128, cl], F32, tag="ps", name="ps")
                for k in range(KT2):
                    nc.tensor.matmul(ps, w2_at(k)[:, m * 128:(m + 1) * 128],
                                     h1c[k][:, :cl],
                                     start=(k == 0), stop=(k == KT2 - 1))
                nc.scalar.activation(out=h2_t[m][:, c0:c0 + cl], func=RELU,
                                     in_=ps, bias=b2_c(m), scale=1.0)
            # prefetch the next chunk's x ahead of the expert-weight triggers
            if ci + 1 < len(echunks):
                xc_next[0] = load_xc(*echunks[ci + 1])
            # run every expert whose segment is fully covered by done chunks
            chunk_end = c0 + cl
            while seg_queue and seg_queue[0][1] + seg_queue[0][2] <= chunk_end:
                e, s0, sl = seg_queue.pop(0)
                emit_expert(e, s0, sl)

        for e, s0, sl in seg_queue:
            emit_expert(e, s0, sl)
        # dec1 weights: sync DMA right behind the last expert weights into the
        # preallocated tile, so the decoder never waits on them
        nc.sync.dma_start(out=dw1_tile,
                          in_=dw1[:].rearrange("(a p) n -> p a n", p=128))
        if pend[0] is not None:
            emit_exp2(pend[0])
            pend[0] = None

        dw1_at = lambda k: dw1_tile[:, k, :]

        p_enc.release()

        # dw2: gpsimd-triggered (its wait on freed encoder space must not
        # block the sync sequencer), streaming during the expert tail.
        p_dec = tc.alloc_tile_pool(name="dec", bufs=1)
        dw2_tiles = []
        for i in range(2):
            t = p_dec.tile([128, KD2 // 2, D_IN], BF16, tag=f"dw2_{i}",
                           name=f"dw2_{i}")
            nc.gpsimd.dma_start(
                out=t,
                in_=dw2[i * 8 * 128:(i + 1) * 8 * 128, :]
                .rearrange("(a p) n -> p a n", p=128))
            dw2_tiles.append(t)
        dw2_at = lambda k: dw2_tiles[k // 8][:, k % 8, :]

        # ---------------- decoder (fused dec1+dec2 per chunk) -----------------
        for c0, cl in dchunks:
            d1c = []
            for m in range(MD1):
                ps = p_ps.tile([128, cl], F32, tag="ps", name="ps")
                for k in range(KD1):
                    nc.tensor.matmul(ps, dw1_at(k)[:, m * 128:(m + 1) * 128],
                                     e2_t[k][:, c0:c0 + cl],
                                     start=(k == 0), stop=(k == KD1 - 1))
                d = p_dec.tile([128, CHUNK], BF16, tag="d1c", name="d1c",
                               bufs=MD1)
                nc.scalar.activation(out=d[:, :cl], in_=ps, func=RELU,
                                     bias=db1_c(m), scale=1.0)
                d1c.append(d)
            for m in range(MD2):
                ps = p_ps.tile([128, cl], F32, tag="ps", name="ps")
                for k in range(KD2):
                    nc.tensor.matmul(ps, dw2_at(k)[:, m * 128:(m + 1) * 128],
                                     d1c[k][:, :cl],
                                     start=(k == 0), stop=(k == KD2 - 1))
                o_t = p_dec.tile([128, CHUNK], F32, tag="o", name="o", bufs=8)
                nc.scalar.activation(out=o_t[:, :cl], in_=ps, func=IDENT,
                                     bias=db2_c(m), scale=1.0)
                nc.sync.dma_start(out=out[m * 128:(m + 1) * 128, c0:c0 + cl],
                                  in_=o_t[:, :cl])

        p_dec.release()
        p_decw.release()
        p_exp.release()
        p_h2.release()
        p_e2.release()
        p_ps.release()
        p_const.release()

    nc.finalize()
    return nc


_CACHE = {}


def _get_nc(n_seg, n_core):
    key = tuple(n_seg)
    if key not in _CACHE:
        _CACHE[key] = _build(n_seg, n_core)
    return _CACHE[key]


def _bf16(a):
    return np.ascontiguousarray(np.asarray(a, np.float32).astype(ml_dtypes.bfloat16))


def _fm(w, kt):
    """[kt*128, n] row-major -> [128, kt, n] feature-major flat [128, kt*n]."""
    a = np.asarray(w, np.float32).astype(ml_dtypes.bfloat16)
    kt128, n = a.shape
    return a.reshape(kt, 128, n).transpose(1, 0, 2).reshape(128, kt * n)


def _pack_bias(b1, b2, Eb1, Eb2, Db1, Db2):
    out = np.zeros((128, B_COLS), np.float32)

    def put(off, vec, mt):
        out[:, off:off + mt] = np.asarray(vec, np.float32).reshape(mt, 128).T

    put(B1_OFF, b1, D_H // 128)
    put(B2_OFF, b2, D_BOT // 128)
    for e in range(N_CLS):
        put(EB1_OFF + e * (D_EXP // 128), Eb1[e], D_EXP // 128)
        put(EB2_OFF + e * (D_BOT // 128), Eb2[e], D_BOT // 128)
    put(DB1_OFF, Db1, D_H // 128)
    put(DB2_OFF, Db2, D_IN // 128)
    return np.ascontiguousarray(out)


def kernel(x, labels, W1, b1, W2, b2, EW1, Eb1, EW2, Eb2, DW1, Db1, DW2, Db2):
    x = np.asarray(x, dtype=np.float32)
    labels_np = np.asarray(labels).astype(np.int64)
    B = x.shape[0]

    counts = np.bincount(labels_np, minlength=N_CLS)
    n_seg = [int(-(-int(c) // N_CORES)) for c in counts]  # ceil(c/8)
    n_core = int(sum(n_seg))

    # assign tokens: class e sorted tokens padded to 8*n_seg[e], row j -> core j
    order = np.argsort(labels_np, kind="stable")
    idx_by_class = np.split(order, np.cumsum(counts)[:-1])
    core_tok = np.full((N_CORES, n_core), -1, dtype=np.int64)
    off = 0
    for e in range(N_CLS):
        ne = n_seg[e]
        if ne == 0:
            continue
        padded = np.full(N_CORES * ne, -1, dtype=np.int64)
        padded[:counts[e]] = idx_by_class[e]
        core_tok[:, off:off + ne] = padded.reshape(N_CORES, ne)
        off += ne

    # packed per-expert weights [N_CLS, 128, 8192]: ew1 feature-major flat
    # [128, 4*1024] then ew2 feature-major flat [128, 8*512]
    ew = np.empty((N_CLS, 128, 8192), ml_dtypes.bfloat16)
    for e in range(N_CLS):
        ew[e, :, :4096] = _fm(EW1[e], D_BOT // 128)
        ew[e, :, 4096:] = _fm(EW2[e], D_EXP // 128)

    weights = {
        "w1": _bf16(W1), "w2": _bf16(W2), "ew": np.ascontiguousarray(ew),
        "dw1": _bf16(DW1), "dw2": _bf16(DW2),
        "bias": _pack_bias(b1, b2, Eb1, Eb2, Db1, Db2),
    }

    x_bf = x.astype(ml_dtypes.bfloat16)
    in_maps = []
    for j in range(N_CORES):
        ids = core_tok[j]
        valid = ids >= 0
        xc = np.zeros((n_core, D_IN), dtype=ml_dtypes.bfloat16)
        xc[valid] = x_bf[ids[valid]]
        im = {"xt": np.ascontiguousarray(xc.T)}
        im.update(weights)
        in_maps.append(im)

    nc = _get_nc(n_seg, n_core)
    res = run_bass_kernel_spmd(nc, in_maps, core_ids=list(range(N_CORES)))

    out = np.empty((B, D_IN), dtype=np.float32)
    for j in range(N_CORES):
        oc = res.results[j]["out"]  # [D_IN, n_core]
        ids = core_tok[j]
        valid = ids >= 0
        out[ids[valid]] = oc.T[valid]
    return out


# revision 14
# speedup vs baseline: 1.0168x; 1.0001x over previous
"""Class-routed autoencoder (moe_routing) Trainium2 kernel.

Strategy:
- The reference computes ALL 10 experts densely then gathers by label; we
  ROUTE instead: sort tokens by class on the host, split every class's tokens
  evenly across the 8 cores (class counts padded up to a multiple of 8 with
  dummy zero tokens), so every core runs an IDENTICAL program (SPMD) on
  N_core = sum_e ceil(c_e/8) tokens laid out as 10 contiguous single-class
  segments. Expert layers slice the right weight block per segment at
  compile time; no gather/scatter on device.
- Everything runs feature-major ([features, tokens]): weights are the
  stationary matmul operand as-is (out = W.T @ x_fm), the batch is the
  moving/free dim, and per-feature bias + ReLU + PSUM->SBUF evacuation fuse
  into one scalar-engine activation op (bias is per-partition).
- Matmuls run in bf16 (fp32 PSUM accumulate, biases added in fp32).
- The kernel is PE-row-bound (~192us of row streaming at 2.4GHz); the
  optimization work is in eliminating non-streaming time:
  * ~16 warmup matmuls on zeroed tiles run during the initial DMA wait so
    the PE p-state ramp (0.65->2.4GHz over ~3us) completes before real data
    arrives.
  * The first encoder chunk (512 cols) runs enc1 k-OUTER over 8 PSUM banks
    per m-half, so the first matmul needs only the first x k-pair (0.26MB)
    + the first half W1 k-slice (0.25MB) instead of x-chunk + 1MB W1 slice.
    W1 is split into 9 DMAs interleaved with the x k-pair DMAs in
    need-order; W2/xc1/xc2 stream behind them.
  * All 6 bias vectors are packed into ONE [128,164] f32 dram tensor (one
    DMA trigger instead of six; the sync sequencer pays ~1.3us per trigger).
  * Each expert's two weight matrices are packed into ONE [128,8192] bf16
    tensor (one trigger per expert), prefetched through a 4-deep ring whose
    slots are allocated below the encoder arena (no false WAR deps).
  * Decoder dw1 is preallocated below the encoder arena and its sync DMA
    triggers right after the last expert weights, so dec1 never waits
    (the baseline had a ~2us PE gap here); dw2 streams via the gpsimd
    sequencer into released encoder space during the expert tail.
  * Decoder chunks are [~450, ~450, 128]: the small last chunk shrinks the
    post-last-matmul activation+output-DMA tail.
- Expert execution is interleaved into the encoder chunk loop: each expert
  runs as soon as the encoder chunks covering its column segment are done,
  so the expert-weight DMA stream hides entirely under encoder compute.
- Host: permute+transpose x, run 8 cores, inverse-permute the output.
"""

import ml_dtypes
import numpy as np

import concourse.bass as bass
import concourse.mybir as mybir
import concourse.tile as tile
from concourse import bacc
from concourse.bass_utils import run_bass_kernel_spmd

N_CORES = 8
N_CLS = 10
D_IN, D_H, D_BOT, D_EXP = 1024, 2048, 512, 1024

F32 = mybir.dt.float32
BF16 = mybir.dt.bfloat16
RELU = mybir.ActivationFunctionType.Relu
IDENT = mybir.ActivationFunctionType.Identity

CHUNK = 512  # max matmul moving-operand (free dim) size: one PSUM bank fp32

# bias_all column layout: [b1(16) b2(4) eb1(10*8) eb2(10*4) db1(16) db2(8)]
B1_OFF, B2_OFF, EB1_OFF, EB2_OFF, DB1_OFF, DB2_OFF, B_COLS = (
    0, 16, 20, 100, 140, 156, 164)


def _chunks(n, step=CHUNK):
    """Balanced split of n into ceil(n/step) near-equal pieces (all <= step)."""
    nch = -(-n // step)
    base, rem = divmod(n, nch)
    out = []
    s = 0
    for i in range(nch):
        sz = base + (1 if i < rem else 0)
        out.append((s, sz))
        s += sz
    return out


def _enc_chunks(n):
    """First chunk maximal (512) for the k-outer startup path, rest balanced."""
    first = min(CHUNK, n)
    out = [(0, first)]
    if n > first:
        out += [(first + s, l) for s, l in _chunks(n - first)]
    return out


def _dec_chunks(n):
    """Small last chunk so the tail activation+output-DMA is short."""
    if n <= 384:
        return _chunks(n)
    tail = 128
    return _chunks(n - tail) + [(n - tail, tail)]


def _build(n_seg, n_core):
    """Build the SPMD program for per-class-per-core counts n_seg (sum=n_core)."""
    nc = bacc.Bacc()

    xt = nc.dram_tensor("xt", [D_IN, n_core], BF16, kind="ExternalInput")
    w1 = nc.dram_tensor("w1", [D_IN, D_H], BF16, kind="ExternalInput")
    w2 = nc.dram_tensor("w2", [D_H, D_BOT], BF16, kind="ExternalInput")
    ew = nc.dram_tensor("ew", [N_CLS, 128, 8192], BF16, kind="ExternalInput")
    dw1 = nc.dram_tensor("dw1", [D_BOT, D_H], BF16, kind="ExternalInput")
    dw2 = nc.dram_tensor("dw2", [D_H, D_IN], BF16, kind="ExternalInput")
    bias = nc.dram_tensor("bias", [128, B_COLS], F32, kind="ExternalInput")
    out = nc.dram_tensor("out", [D_IN, n_core], F32, kind="ExternalOutput")

    segs = []  # (class e, col start, col len)
    s = 0
    for e in range(N_CLS):
        if n_seg[e] > 0:
            segs.append((e, s, n_seg[e]))
            s += n_seg[e]
    echunks = _enc_chunks(n_core)
    dchunks = _dec_chunks(n_core)

    KT1, MT1 = D_IN // 128, D_H // 128     # enc1: 8, 16
    KT2, MT2 = D_H // 128, D_BOT // 128    # enc2: 16, 4
    KE1, ME1 = D_BOT // 128, D_EXP // 128  # exp1: 4, 8
    KE2, ME2 = D_EXP // 128, D_BOT // 128  # exp2: 8, 4
    KD1, MD1 = D_BOT // 128, D_H // 128    # dec1: 4, 16
    KD2, MD2 = D_H // 128, D_IN // 128     # dec2: 16, 8

    with tile.TileContext(nc) as tc:
        p_const = tc.alloc_tile_pool(name="const", bufs=1)
        p_ps = tc.alloc_tile_pool(name="ps", bufs=8, space="PSUM")

        warm_w = p_const.tile([128, 128], BF16, tag="warmw", name="warmw")

        # bottleneck activations, SBUF-resident at full width
        p_e2 = tc.alloc_tile_pool(name="e2", bufs=1)
        p_h2 = tc.alloc_tile_pool(name="h2", bufs=1)
        e2_t = [p_e2.tile([128, n_core], BF16, tag=f"e2_{m}", name=f"e2_{m}")
                for m in range(D_BOT // 128)]
        h2_t = [p_h2.tile([128, n_core], BF16, tag=f"h2_{m}", name=f"h2_{m}")
                for m in range(D_BOT // 128)]

        # Expert + decoder-weight arenas are allocated BEFORE the encoder
        # pool: their space never overlaps encoder tiles, so their DMAs carry
        # no false WAR deps and prefetch during the encoder phase.
        EW_BUFS = 4
        ECHUNK = 256
        p_exp = tc.alloc_tile_pool(name="exp", bufs=1)
        e1_ring = [p_exp.tile([128, D_EXP // 128, ECHUNK], BF16, tag=f"e1r_{i}",
                              name=f"e1r_{i}") for i in range(3)]
        # packed per-expert weights: cols [0:4096]=ew1 (k,1024), [4096:8192]=ew2
        ew_ring = [p_exp.tile([128, 8192], BF16, tag=f"ew_{i}", name=f"ew_{i}")
                   for i in range(EW_BUFS)]
        p_decw = tc.alloc_tile_pool(name="decw", bufs=1)
        dw1_tile = p_decw.tile([128, KD1, D_H], BF16, tag="dw1", name="dw1")

        b_t = p_const.tile([128, B_COLS], F32, tag="bias", name="bias")

        # ---------------- encoder ----------------
        p_enc = tc.alloc_tile_pool(name="enc", bufs=1)

        # x chunk 0 as 4 k-pair DMAs so the first matmul waits on 0.26MB only
        c0_start, c0_len = echunks[0]
        xc0 = p_enc.tile([128, KT1, CHUNK], BF16, tag="xc", name="xc", bufs=3)

        def load_xc0_pair(j, eng):
            eng.dma_start(
                out=xc0[:, 2 * j:2 * j + 2, :c0_len],
                in_=xt[j * 256:(j + 1) * 256, c0_start:c0_start + c0_len]
                .rearrange("(a p) n -> p a n", p=128))

        # W1 k0 split into four column quarters (m-groups of 4), rest per-k
        w1k0 = [p_enc.tile([128, D_H // 4], BF16, tag=f"w1k0q{q}",
                           name=f"w1k0q{q}") for q in range(4)]
        w1k = [None] * KT1

        def load_w1k0_q(q, eng):
            eng.dma_start(
                out=w1k0[q],
                in_=w1[0:128, q * (D_H // 4):(q + 1) * (D_H // 4)])

        def load_w1k(k):
            t = p_enc.tile([128, D_H], BF16, tag=f"w1k{k}", name=f"w1k{k}")
            nc.sync.dma_start(out=t, in_=w1[k * 128:(k + 1) * 128, :])
            w1k[k] = t

        def w1_at(k, m):
            if k == 0:
                return w1k0[m // 4][:, (m % 4) * 128:(m % 4 + 1) * 128]
            return w1k[k][:, m * 128:(m + 1) * 128]

        def load_xc0_plane(k, eng):
            eng.dma_start(out=xc0[:, k, :c0_len],
                          in_=xt[k * 128:(k + 1) * 128,
                                 c0_start:c0_start + c0_len])

        # Startup DMA schedule in need-order. The engines leave their common
        # start barrier at ~6.8us and every DMA trigger costs ~0.7us of
        # sequencer time, so the critical-path pieces (first W1 k0 quarter +
        # first x plane: 0.26MB) are spread over the scalar/gpsimd/sync
        # queues in parallel.
        load_w1k0_q(0, nc.scalar)       # W1 k0, m0-3 cols
        load_xc0_plane(0, nc.gpsimd)    # x plane 0 -> enc1 k=0
        load_w1k0_q(1, nc.scalar)       # W1 k0, m4-7 cols
        load_xc0_plane(1, nc.gpsimd)    # x plane 1 -> enc1 k=1
        load_w1k0_q(2, nc.sync)
        load_w1k0_q(3, nc.sync)
        load_xc0_pair(1, nc.sync)       # planes 2-3
        load_w1k(1)
        load_w1k(2)
        load_xc0_pair(2, nc.sync)
        load_w1k(3)
        nc.sync.dma_start(out=b_t, in_=bias[:])
        load_xc0_pair(3, nc.sync)
        load_w1k(4)
        load_w1k(5)
        load_w1k(6)
        load_w1k(7)

        # ---- PE warmup: ramp the clock during the initial DMA wait ----
        # Tiny matmuls keep the PE busy until the first real operands land,
        # so the p-state reaches 2.4GHz before real work starts instead of
        # ramping through it.
        nc.vector.memset(warm_w, 0.0)
        for _ in range(38):
            ps = p_ps.tile([128, 128], F32, tag="ps", name="ps")
            nc.tensor.matmul(ps, warm_w, warm_w, start=True, stop=True)

        def load_w(dram_h, pool, tag, kt, mt_cols, ksplit):
            tiles = []
            per = kt // ksplit
            for i in range(ksplit):
                t = pool.tile([128, per, mt_cols], BF16, tag=f"{tag}{i}",
                              name=f"{tag}{i}")
                nc.sync.dma_start(
                    out=t,
                    in_=dram_h[i * per * 128:(i + 1) * per * 128, :]
                    .rearrange("(a p) n -> p a n", p=128))
                tiles.append(t)
            return lambda k: tiles[k // per][:, k % per, :]

        w2_at = load_w(w2, p_enc, "w2_", KT2, D_BOT, 2)

        def load_xc(c0, cl):
            t = p_enc.tile([128, KT1, CHUNK], BF16, tag="xc", name="xc", bufs=3)
            nc.sync.dma_start(
                out=t[:, :, :cl],
                in_=xt[:, c0:c0 + cl].rearrange("(a p) n -> p a n", p=128))
            return t

        # bias slice helpers (per-partition [128,1] APs into the packed tile)
        b1_c = lambda m: b_t[:, B1_OFF + m:B1_OFF + m + 1]
        b2_c = lambda m: b_t[:, B2_OFF + m:B2_OFF + m + 1]
        eb1_c = lambda e, m: b_t[:, EB1_OFF + e * ME1 + m:EB1_OFF + e * ME1 + m + 1]
        eb2_c = lambda e, m: b_t[:, EB2_OFF + e * ME2 + m:EB2_OFF + e * ME2 + m + 1]
        db1_c = lambda m: b_t[:, DB1_OFF + m:DB1_OFF + m + 1]
        db2_c = lambda m: b_t[:, DB2_OFF + m:DB2_OFF + m + 1]

        # experts are emitted as soon as the encoder chunks covering their
        # column segment are done: their compute absorbs expert-weight DMA
        # latency, and the PE never waits on the weight stream at phase end.
        seg_queue = list(segs)
        exp_counter = [0]
        unit_ctr = [0]
        pend = [None]  # exp2 of each unit is delayed one unit behind its exp1

        def emit_exp1(u):
            e, a, al, slot, ew_t = u
            e1c = e1_ring[slot]
            for m in range(ME1):
                ps = p_ps.tile([128, al], F32, tag="ps", name="ps")
                for k in range(KE1):
                    nc.tensor.matmul(
                        ps,
                        ew_t[:, k * D_EXP + m * 128:k * D_EXP + (m + 1) * 128],
                        h2_t[k][:, a:a + al],
                        start=(k == 0), stop=(k == KE1 - 1))
                # bias+relu on the (idle) vector engine: keeps PSUM
                # evacuation off the scalar engine's critical path
                nc.vector.tensor_scalar(
                    out=e1c[:, m, :al], in0=ps,
                    scalar1=eb1_c(e, m), scalar2=0.0,
                    op0=mybir.AluOpType.add, op1=mybir.AluOpType.max)

        def emit_exp2(u):
            e, a, al, slot, ew_t = u
            e1c = e1_ring[slot]
            for m in range(ME2):
                ps = p_ps.tile([128, al], F32, tag="ps", name="ps")
                for k in range(KE2):
                    nc.tensor.matmul(
                        ps,
                        ew_t[:, 4096 + k * D_BOT + m * 128:
                             4096 + k * D_BOT + (m + 1) * 128],
                        e1c[:, k, :al],
                        start=(k == 0), stop=(k == KE2 - 1))
                nc.scalar.activation(out=e2_t[m][:, a:a + al], in_=ps,
                                     func=RELU, bias=eb2_c(e, m), scale=1.0)

        def emit_expert(e, s0, sl):
            # exp1(unit i) then exp2(unit i-1): exp1's PSUM evacuations (DVE)
            # overlap the next unit's exp1 matmuls instead of stalling the PE
            ei = exp_counter[0]
            exp_counter[0] += 1
            ew_t = ew_ring[ei % EW_BUFS]
            nc.sync.dma_start(out=ew_t, in_=ew[e])
            for c0, cl in _chunks(sl, ECHUNK):
                u = (e, s0 + c0, cl, unit_ctr[0] % 3, ew_t)
                unit_ctr[0] += 1
                emit_exp1(u)
                if pend[0] is not None:
                    emit_exp2(pend[0])
                pend[0] = u

        xc_next = [None]
        for ci, (c0, cl) in enumerate(echunks):
            xc = xc_next[0]
            if ci == 0:
                # enc1 k-OUTER over two m-halves of 8 PSUM banks each: the
                # first matmul needs only the first x k-pair + half W1 k0
                h1c = []
                for half in range(2):
                    pss = [p_ps.tile([128, cl], F32, tag="ps", name="ps")
                           for _ in range(8)]
                    for k in range(KT1):
                        for mi in range(8):
                            nc.tensor.matmul(
                                pss[mi], w1_at(k, half * 8 + mi),
                                xc0[:, k, :cl],
                                start=(k == 0), stop=(k == KT1 - 1))
                    for mi in range(8):
                        m = half * 8 + mi
                        h = p_enc.tile([128, CHUNK], BF16, tag="h1c",
                                       name="h1c", bufs=MT1)
                        nc.scalar.activation(out=h[:, :cl], in_=pss[mi],
                                             func=RELU, bias=b1_c(m),
                                             scale=1.0)
                        h1c.append(h)
            else:
                h1c = []
                for m in range(MT1):
                    ps = p_ps.tile([128, cl], F32, tag="ps", name="ps")
                    for k in range(KT1):
                        nc.tensor.matmul(ps, w1_at(k, m), xc[:, k, :cl],
                                         start=(k == 0), stop=(k == KT1 - 1))
                    h = p_enc.tile([128, CHUNK], BF16, tag="h1c", name="h1c",
                                   bufs=MT1)
                    nc.scalar.activation(out=h[:, :cl], in_=ps, func=RELU,
                                         bias=b1_c(m), scale=1.0)
                    h1c.append(h)
            for m in range(MT2):
                ps = p_ps.tile([128, cl], F32, tag="ps", name="ps")
                for k in range(KT2):
                    nc.tensor.matmul(ps, w2_at(k)[:, m * 128:(m + 1) * 128],
                                     h1c[k][:, :cl],
                                     start=(k == 0), stop=(k == KT2 - 1))
                nc.scalar.activation(out=h2_t[m][:, c0:c0 + cl], func=RELU,
                                     in_=ps, bias=b2_c(m), scale=1.0)
            # prefetch the next chunk's x ahead of the expert-weight triggers
            if ci + 1 < len(echunks):
                xc_next[0] = load_xc(*echunks[ci + 1])
            # run every expert whose segment is fully covered by done chunks
            chunk_end = c0 + cl
            while seg_queue and seg_queue[0][1] + seg_queue[0][2] <= chunk_end:
                e, s0, sl = seg_queue.pop(0)
                emit_expert(e, s0, sl)

        for e, s0, sl in seg_queue:
            emit_expert(e, s0, sl)
        # dec1 weights: sync DMA right behind the last expert weights into the
        # preallocated tile, so the decoder never waits on them
        nc.sync.dma_start(out=dw1_tile,
                          in_=dw1[:].rearrange("(a p) n -> p a n", p=128))
        if pend[0] is not None:
            emit_exp2(pend[0])
            pend[0] = None

        dw1_at = lambda k: dw1_tile[:, k, :]

        p_enc.release()

        # dw2: gpsimd-triggered (its wait on freed encoder space must not
        # block the sync sequencer), streaming during the expert tail.
        p_dec = tc.alloc_tile_pool(name="dec", bufs=1)
        dw2_tiles = []
        for i in range(2):
            t = p_dec.tile([128, KD2 // 2, D_IN], BF16, tag=f"dw2_{i}",
                           name=f"dw2_{i}")
            nc.gpsimd.dma_start(
                out=t,
                in_=dw2[i * 8 * 128:(i + 1) * 8 * 128, :]
                .rearrange("(a p) n -> p a n", p=128))
            dw2_tiles.append(t)
        dw2_at = lambda k: dw2_tiles[k // 8][:, k % 8, :]

        # ---------------- decoder (fused dec1+dec2 per chunk) -----------------
        last_c0 = dchunks[-1][0]
        for c0, cl in dchunks:
            d1c = []
            for m in range(MD1):
                ps = p_ps.tile([128, cl], F32, tag="ps", name="ps")
                for k in range(KD1):
                    nc.tensor.matmul(ps, dw1_at(k)[:, m * 128:(m + 1) * 128],
                                     e2_t[k][:, c0:c0 + cl],
                                     start=(k == 0), stop=(k == KD1 - 1))
                d = p_dec.tile([128, CHUNK], BF16, tag="d1c", name="d1c",
                               bufs=MD1)
                nc.scalar.activation(out=d[:, :cl], in_=ps, func=RELU,
                                     bias=db1_c(m), scale=1.0)
                d1c.append(d)
            for m in range(MD2):
                ps = p_ps.tile([128, cl], F32, tag="ps", name="ps")
                for k in range(KD2):
                    nc.tensor.matmul(ps, dw2_at(k)[:, m * 128:(m + 1) * 128],
                                     d1c[k][:, :cl],
                                     start=(k == 0), stop=(k == KD2 - 1))
                o_t = p_dec.tile([128, CHUNK], F32, tag="o", name="o", bufs=8)
                if c0 == last_c0 and m % 2 == 1:
                    # final chunk: alternate evacuation onto the idle DVE so
                    # the post-last-matmul activation chain halves in length
                    nc.vector.tensor_scalar_add(out=o_t[:, :cl], in0=ps,
                                                scalar1=db2_c(m))
                else:
                    nc.scalar.activation(out=o_t[:, :cl], in_=ps, func=IDENT,
                                         bias=db2_c(m), scale=1.0)
                nc.sync.dma_start(out=out[m * 128:(m + 1) * 128, c0:c0 + cl],
                                  in_=o_t[:, :cl])

        p_dec.release()
        p_decw.release()
        p_exp.release()
        p_h2.release()
        p_e2.release()
        p_ps.release()
        p_const.release()

    nc.finalize()
    return nc


_CACHE = {}


def _get_nc(n_seg, n_core):
    key = tuple(n_seg)
    if key not in _CACHE:
        _CACHE[key] = _build(n_seg, n_core)
    return _CACHE[key]


def _bf16(a):
    return np.ascontiguousarray(np.asarray(a, np.float32).astype(ml_dtypes.bfloat16))


def _fm(w, kt):
    """[kt*128, n] row-major -> [128, kt, n] feature-major flat [128, kt*n]."""
    a = np.asarray(w, np.float32).astype(ml_dtypes.bfloat16)
    kt128, n = a.shape
    return a.reshape(kt, 128, n).transpose(1, 0, 2).reshape(128, kt * n)


def _pack_bias(b1, b2, Eb1, Eb2, Db1, Db2):
    out = np.zeros((128, B_COLS), np.float32)

    def put(off, vec, mt):
        out[:, off:off + mt] = np.asarray(vec, np.float32).reshape(mt, 128).T

    put(B1_OFF, b1, D_H // 128)
    put(B2_OFF, b2, D_BOT // 128)
    for e in range(N_CLS):
        put(EB1_OFF + e * (D_EXP // 128), Eb1[e], D_EXP // 128)
        put(EB2_OFF + e * (D_BOT // 128), Eb2[e], D_BOT // 128)
    put(DB1_OFF, Db1, D_H // 128)
    put(DB2_OFF, Db2, D_IN // 128)
    return np.ascontiguousarray(out)


def kernel(x, labels, W1, b1, W2, b2, EW1, Eb1, EW2, Eb2, DW1, Db1, DW2, Db2):
    x = np.asarray(x, dtype=np.float32)
    labels_np = np.asarray(labels).astype(np.int64)
    B = x.shape[0]

    counts = np.bincount(labels_np, minlength=N_CLS)
    n_seg = [int(-(-int(c) // N_CORES)) for c in counts]  # ceil(c/8)
    n_core = int(sum(n_seg))

    # assign tokens: class e sorted tokens padded to 8*n_seg[e], row j -> core j
    order = np.argsort(labels_np, kind="stable")
    idx_by_class = np.split(order, np.cumsum(counts)[:-1])
    core_tok = np.full((N_CORES, n_core), -1, dtype=np.int64)
    off = 0
    for e in range(N_CLS):
        ne = n_seg[e]
        if ne == 0:
            continue
        padded = np.full(N_CORES * ne, -1, dtype=np.int64)
        padded[:counts[e]] = idx_by_class[e]
        core_tok[:, off:off + ne] = padded.reshape(N_CORES, ne)
        off += ne

    # packed per-expert weights [N_CLS, 128, 8192]: ew1 feature-major flat
    # [128, 4*1024] then ew2 feature-major flat [128, 8*512]
    ew = np.empty((N_CLS, 128, 8192), ml_dtypes.bfloat16)
    for e in range(N_CLS):
        ew[e, :, :4096] = _fm(EW1[e], D_BOT // 128)
        ew[e, :, 4096:] = _fm(EW2[e], D_EXP // 128)

    weights = {
        "w1": _bf16(W1), "w2": _bf16(W2), "ew": np.ascontiguousarray(ew),
        "dw1": _bf16(DW1), "dw2": _bf16(DW2),
        "bias": _pack_bias(b1, b2, Eb1, Eb2, Db1, Db2),
    }

    x_bf = x.astype(ml_dtypes.bfloat16)
    in_maps = []
    for j in range(N_CORES):
        ids = core_tok[j]
        valid = ids >= 0
        xc = np.zeros((n_core, D_IN), dtype=ml_dtypes.bfloat16)
        xc[valid] = x_bf[ids[valid]]
        im = {"xt": np.ascontiguousarray(xc.T)}
        im.update(weights)
        in_maps.append(im)

    nc = _get_nc(n_seg, n_core)
    res = run_bass_kernel_spmd(nc, in_maps, core_ids=list(range(N_CORES)))

    out = np.empty((B, D_IN), dtype=np.float32)
    for j in range(N_CORES):
        oc = res.results[j]["out"]  # [D_IN, n_core]
        ids = core_tok[j]
        valid = ids >= 0
        out[ids[valid]] = oc.T[valid]
    return out
